# revision 25
# baseline (speedup 1.0000x reference)
"""Trainium2 Bass kernel for nn_MidAttnBlock (res-block -> full LxL attention -> res-block).

Contract: kernel(**inputs) takes the FULL inputs of reference.setup_inputs()
(x: (16,256,2048) f32, t: (16,256,1) f32, plus conv/groupnorm/linear params)
and returns the FULL (16,256,2048) f32 output.  Data-parallel over batch on
8 NeuronCores, 2 samples per core; each core runs an identical Bass program.

Convs and the kqv projection run in bf16 (full-rate PE, half the SBUF/HBM
of f32r).  The attention (scores, softmax weights, A@V) runs in fp8e4 with
DoubleRow matmuls (256-deep contraction per PE pass).  ~7.5e-3 end-to-end
relative error (threshold 2e-2).  exp is computed as exp(s/16 - 4) so the
softmax weights fit fp8e4's range; the shift cancels in the normalization.

The groupnorm relu applies run on the Vector engine as one
tensor_scalar((x*s) max -b2) per chunk: the activation tiles store
relu(gn(x)) - b2 with pad columns = -b2, and the following conv adds back
the per-output-channel constant corr = Wsum @ b2 (folded into the t-vector
add for conv1 and the residual add for conv2).  This keeps ScalarE free for
the attention exps and the kqv PSUM drains.

The two samples on each core are issued interleaved
(s0:r1,kqv | s1:r1 | s0:attn | s1:kqv | s0:r2 | s1:attn | s1:r2) so the
groupnorm stat chains and softmax tails of one sample overlap the other
sample's matmuls.

Self-contained: all shapes/sharding hardcoded.
"""

import json as _json

import ml_dtypes
import numpy as np

import concourse.bass as bass
import concourse.bass2jax as _b2j
import concourse.bass_utils as _bu
import concourse.tile as tile
from concourse import mybir
from concourse.vector_clock import ScopedClock, VectorClock


def _split_bir_waits(bir_json):
    """The walrus_driver in this container encodes at most ONE sync-wait per
    instruction (and none on Drain).  Tile's sem assigner attaches several.
    Rewrite the BIR: excess waits move to single-wait NoOps inserted directly
    before the instruction on the same engine."""
    m = _json.loads(bir_json)
    ctr = 0
    for fn in m.get("functions", []):
        for bb in fn.get("blocks", []):
            out = []
            for ins in bb.get("instructions", []):
                si = ins.get("sync_info")
                waits = (si or {}).get("on_wait") or []
                keep = 0 if ins.get("opcode") == "Drain" else 1
                if len(waits) > keep:
                    nmove = len(waits) - keep
                    for w in waits[:nmove]:
                        ctr += 1
                        out.append({
                            "debug": ins.get("debug", 0),
                            "engine": ins["engine"],
                            "ins": [],
                            "name": f"{ins['name']}-wsp{ctr}",
                            "opcode": "NoOp",
                            "outs": [],
                            "sync_info": {"on_update": [], "on_wait": [w]},
                        })
                    si["on_wait"] = waits[nmove:]
                out.append(ins)
            bb["instructions"] = out
    return _json.dumps(m).encode()


_orig_compile_bir_kernel = _bu.compile_bir_kernel


def _compile_bir_splitwaits(bir_json, tmpdir, neff_name="file.neff"):
    return _orig_compile_bir_kernel(_split_bir_waits(bir_json), tmpdir, neff_name)


if getattr(_bu.compile_bir_kernel, "__name__", "") != "_compile_bir_splitwaits":
    _bu.compile_bir_kernel = _compile_bir_splitwaits
    _b2j.compile_bir_kernel = _compile_bir_splitwaits


F32 = mybir.dt.float32
F32R = mybir.dt.float32r
BF16 = mybir.dt.bfloat16
FP8 = mybir.dt.float8e4
AF = mybir.ActivationFunctionType
OP = mybir.AluOpType
DR = mybir.MatmulPerfMode.DoubleRow

P = 128          # partitions
C = 256          # channels
CB = 2           # channel blocks of 128
L = 2048         # sequence length
LS = 512         # l-slice (matmul moving dim)
NL = L // LS     # 4 slices
KB = L // P      # 16 k-blocks for attention
NP = KB // 2     # 8 k-block pairs (DoubleRow)
GPB = 16         # groups per channel-block (32 groups, 8 ch each)
EPS = 1e-5
S = 2            # samples per core
NCORES = 8
SCALE = 1.0 / 16.0   # 1/sqrt(C)
ESHIFT = -4.0        # exp(s*SCALE + ESHIFT): keeps softmax weights in fp8e4 range


class _TileContextPatched(tile.TileContext):
    """TileContext whose kernel-tail drain carries no sem waits (the container
    walrus rejects waits on Drain); one SP NOP per proc carries them instead."""

    def _drain_and_barrier(self, tick_clock, wait_clock):
        gc = tick_clock.global_clock
        n = len(gc)
        for p in range(n):
            v = gc[p]
            if v > 0:
                vec = [0] * n
                vec[p] = v
                nop = self.nc.sync.nop()
                wait_clock.add_sem_waits(nop.ins, ScopedClock({None: VectorClock(vec)}))
        self.nc.sync.drain()
        self.nc.all_engine_barrier()
        assert self.sems is not None
        popped = self.nc._tile_sem_poison_stack.pop()
        assert popped is self._sem_poison
        self.nc.clear_and_free_semaphores(list(self.sems.allocated().values()))
        self.nc.all_engine_barrier()


def _f(ap):
    """Read an f32r tile as plain f32 (same bits) for VectorE/ScalarE inputs."""
    return ap.bitcast(F32)


def build_program(samples=S, use_bias=()):
    """Build the per-core Bass program (identical on all cores).

    use_bias: subset of {"c2b_r1", "c2b_r2", "linb"} enabling extra adds for
    biases that setup_inputs() keeps at zero.
    """
    nc = bass.Bass()

    # ---- DRAM I/O (per core) ----
    x_d = nc.dram_tensor("x", (samples, C, L), F32R, kind="ExternalInput")
    # t + conv1 bias, host-packed [samples, P, CB, 2(resblock)]
    t_d = nc.dram_tensor("tv", (samples, P, CB, 2), F32, kind="ExternalInput")
    w_conv = {}
    for rb in ("r1", "r2"):
        # host-packed [P(ic within block), icb, tap, oc]
        w_conv[rb, 1] = nc.dram_tensor(f"{rb}_w1t", (P, CB, 3, C), BF16, kind="ExternalInput")
        w_conv[rb, 2] = nc.dram_tensor(f"{rb}_w2t", (P, CB, 3, C), BF16, kind="ExternalInput")
    wkqv_d = nc.dram_tensor("wkqvt", (P, CB, 3 * C), BF16, kind="ExternalInput")
    # tap-summed conv weights for the groupnorm-bias correction:
    # wsum[icp, icb, j, oc] = sum_tap w_j[oc, ic, tap], j in (r1c1, r1c2, r2c1, r2c2)
    wsum_d = nc.dram_tensor("wsumt", (P, CB, 4, C), F32R, kind="ExternalInput")
    # all 8 groupnorm affine vectors in one tensor: [P, CB, rb*4+(ln-1)*2+wb]
    gnall_d = nc.dram_tensor("gnallt", (P, CB, 8), F32, kind="ExternalInput")
    c2b_d = {}
    if "c2b_r1" in use_bias:
        c2b_d["r1"] = nc.dram_tensor("r1_c2bs", (P, CB), F32, kind="ExternalInput")
    if "c2b_r2" in use_bias:
        c2b_d["r2"] = nc.dram_tensor("r2_c2bs", (P, CB), F32, kind="ExternalInput")
    linb_d = None
    if "linb" in use_bias:
        linb_d = nc.dram_tensor("lin_bs", (P, 3 * CB), F32, kind="ExternalInput")
    gind_d = nc.dram_tensor("gind", (P, GPB), F32R, kind="ExternalInput")  # 1/8 group indicator
    bind_d = nc.dram_tensor("bind", (CB, P, P), F32R, kind="ExternalInput")    # group->channel broadcast
    ones8_d = nc.dram_tensor("ones8", (P, CB, 16), FP8, kind="ExternalInput")
    onesr_d = nc.dram_tensor("onesr", (1, P), F32R, kind="ExternalInput")
    out_d = nc.dram_tensor("out", (samples, C, L), F32, kind="ExternalOutput")

    with _TileContextPatched(nc) as tc, \
         tc.tile_pool(name="consts", bufs=1) as consts, \
         tc.tile_pool(name="padp", bufs=4) as padp, \
         tc.tile_pool(name="actp", bufs=2) as actp, \
         tc.tile_pool(name="seqp", bufs=2) as seqp, \
         tc.tile_pool(name="vtp", bufs=1) as vtp, \
         tc.tile_pool(name="expp", bufs=3) as expp, \
         tc.tile_pool(name="outp", bufs=2) as outp, \
         tc.tile_pool(name="rdbp", bufs=2) as rdbp, \
         tc.tile_pool(name="rdsp", bufs=2) as rdsp, \
         tc.tile_pool(name="small", bufs=4) as small, \
         tc.tile_pool(name="persl", bufs=12) as persl, \
         tc.tile_pool(name="t2p", bufs=2) as t2p, \
         tc.tile_pool(name="pacc", bufs=3, space="PSUM") as pacc, \
         tc.tile_pool(name="psc", bufs=2, space="PSUM") as psc, \
         tc.tile_pool(name="paux", bufs=1, space="PSUM") as paux:

        # ---- persistent constants / weights in SBUF ----
        # All const loads ride the gpsimd SWDGE queue (descriptor gen is
        # ~0.6us each, so the count is kept low and ordered so the tensors
        # gating the pipeline head land first); x/t ride the fast ScalarE
        # HWDGE queue concurrently.
        gind_sb = consts.tile([P, GPB], F32R, tag="gind", name="gind")
        nc.gpsimd.dma_start(gind_sb[:], gind_d[:])
        gnall_sb = consts.tile([P, CB, 8], F32, tag="gnall", name="gnall")
        nc.gpsimd.dma_start(gnall_sb[:], gnall_d[:])
        wsum_sb = consts.tile([P, CB, 4, C], F32R, tag="wsum", name="wsum")
        nc.gpsimd.dma_start(wsum_sb[:, :, 0:2, :], wsum_d[:, :, 0:2, :])
        bind_sb = consts.tile([P, CB, P], F32R, tag="bind", name="bind")
        nc.gpsimd.dma_start(bind_sb[:], bind_d.rearrange("cb p c -> p cb c"))

        def gnp(rb, ln, wb, cb):
            idx = (0 if rb == "r1" else 4) + (ln - 1) * 2 + (0 if wb == "w" else 1)
            return gnall_sb[:, cb, idx : idx + 1]

        w1_sb = {}
        w2_sb = {}
        late_consts = []
        for rb in ("r1", "r2"):
            w1_sb[rb] = consts.tile([P, CB, 3, C], BF16, tag=f"w1_{rb}", name=f"w1_{rb}")
            w2_sb[rb] = consts.tile([P, CB, 3, C], BF16, tag=f"w2_{rb}", name=f"w2_{rb}")
            if rb == "r1":
                nc.gpsimd.dma_start(w1_sb[rb][:], w_conv[rb, 1][:])
                late_consts.insert(0, (w2_sb[rb][:], w_conv[rb, 2][:]))
            else:
                late_consts.append((w1_sb[rb][:], w_conv[rb, 1][:]))
                late_consts.append((w2_sb[rb][:], w_conv[rb, 2][:]))
        wkqv_sb = consts.tile([P, CB, 3 * C], BF16, tag="wkqv", name="wkqv")
        late_consts.insert(1, (wkqv_sb[:], wkqv_d[:]))
        late_consts.append((wsum_sb[:, :, 2:4, :], wsum_d[:, :, 2:4, :]))
        c2b_sb = {}
        for rb, d in c2b_d.items():
            c2b_sb[rb] = consts.tile([P, CB], F32, tag=f"c2b_{rb}", name=f"c2b_{rb}")
            late_consts.append((c2b_sb[rb][:], d[:]))
        linb_sb = None
        if linb_d is not None:
            linb_sb = consts.tile([P, 3 * CB], F32, tag="linb", name="linb")
            late_consts.append((linb_sb[:], linb_d[:]))
        ones8_sb = consts.tile([P, CB, 16], FP8, tag="ones8", name="ones8")
        late_consts.append((ones8_sb[:], ones8_d[:]))
        onesr_sb = consts.tile([1, P], F32R, tag="onesr", name="onesr")
        late_consts.append((onesr_sb[:], onesr_d[:]))

        def load_late_consts():
            # issued after the x loads so they don't contend for HBM bandwidth
            # ahead of the first groupnorm
            for ap, src in late_consts:
                nc.gpsimd.dma_start(ap, src)
        eps_sb = consts.tile([P, 1], F32, tag="eps", name="eps")
        nc.vector.memset(eps_sb[:], EPS)
        shift_sb = consts.tile([P, 1], F32, tag="shift", name="shift")
        nc.vector.memset(shift_sb[:], ESHIFT)
        zero2 = consts.tile([P, 2], F32, tag="zero2", name="zero2")
        nc.vector.memset(zero2[:], 0.0)

        def warmup_pe():
            # ~3us of tiny back-to-back matmuls while the x DMAs land, so the
            # PE p-state is at full clock when the first conv group issues
            wk = paux.tile([P, LS], F32, tag="aux", name="warm")
            for _ in range(32):
                nc.tensor.matmul(wk[:GPB, 0:GPB], gind_sb[:], gind_sb[:], start=True, stop=True)

        def alloc_padded(tag, pool, dt=F32R):
            """[P, L+2] tile per channel block; data cols [1, L+1).

            Edge columns are only meaningful for conv sources (the gn_relu
            destinations), where gn_relu writes them to -b2."""
            return [
                pool.tile([P, L + 2], dt, tag=f"{tag}{cb}", name=f"{tag}{cb}")
                for cb in range(CB)
            ]

        def gn_relu(src, dst, rb, ln):
            """dst = relu(groupnorm(src)*w + b) - b2, with pad cols = -b2.

            b2 is the effective per-channel bias (b - m*rstd*w); subtracting it
            turns the ScalarE relu into one DVE tensor_scalar (x*s max -b2),
            and padding with -b2 keeps the following conv exact up to a
            per-output-channel constant corr = Wsum @ b2 that the conv
            consumer adds back (returned here as a [P, CB] column tile)."""
            gp = paux.tile([P, LS], F32, tag="aux", name="gbc")
            for cb in range(CB):
                stats = small.tile([P, NL, 6], F32, tag="stats", name="stats")
                for i in range(NL):
                    nc.vector.bn_stats(out=stats[:, i, :], in_=_f(src[cb][:, 1 + i * LS : 1 + (i + 1) * LS]))
                mv = small.tile([P, 2], F32, tag="mv", name="mv")
                nc.vector.bn_aggr(out=mv[:], in_=stats[:])
                # tmp = [mean_c, E[x^2]_c]  (f32r: feeds the aggregation matmul)
                tmp = small.tile([P, 2], F32R, tag="tmp", name="tmp")
                nc.vector.tensor_copy(out=tmp[:, 0:1], in_=mv[:, 0:1])
                nc.vector.tensor_tensor(out=tmp[:, 1:2], in0=mv[:, 0:1], in1=mv[:, 0:1], op=OP.mult)
                nc.vector.tensor_tensor(out=tmp[:, 1:2], in0=_f(tmp[:, 1:2]), in1=mv[:, 1:2], op=OP.add)
                nc.tensor.matmul(gp[:GPB, 2 * cb : 2 * cb + 2], gind_sb[:], tmp[:], start=True, stop=True)
            # merged group stats; block-cb groups live at partition offset 32*cb
            NG = 32 * CB
            gs = small.tile([NG, 2], F32, tag="gs", name="gs")
            nc.vector.tensor_copy(out=gs[:], in_=zero2[:NG])
            for cb in range(CB):
                nc.vector.tensor_copy(out=gs[cb * 32 : cb * 32 + GPB, :], in_=gp[:GPB, 2 * cb : 2 * cb + 2])
            var = small.tile([NG, 1], F32, tag="var", name="var")
            nc.vector.tensor_tensor(out=var[:], in0=gs[:, 0:1], in1=gs[:, 0:1], op=OP.mult)
            nc.vector.tensor_tensor(out=var[:], in0=gs[:, 1:2], in1=var[:], op=OP.subtract)
            nc.scalar.activation(out=var[:], in_=var[:], func=AF.Ln, bias=eps_sb[:NG])
            rstd = small.tile([NG, 1], F32, tag="rstd", name="rstd")
            nc.scalar.activation(out=rstd[:], in_=var[:], func=AF.Exp, scale=-0.5)
            # pack [rstd_g, +m_g], zero-extended to 128 partitions
            gpk = small.tile([P, 2], F32R, tag="gpk", name="gpk")
            nc.vector.tensor_copy(out=gpk[:], in_=zero2[:])
            nc.vector.tensor_copy(out=gpk[:NG, 0:1], in_=rstd[:])
            nc.vector.tensor_copy(out=gpk[:NG, 1:2], in_=gs[:, 0:1])
            sbs = []
            for cb in range(CB):
                # broadcast to channels: bc[c, :] = [rstd_g(c), m_g(c)]
                nc.tensor.matmul(gp[:, 4 + 2 * cb : 6 + 2 * cb], bind_sb[:, cb, :], gpk[:], start=True, stop=True)
                # sb = [s, -b2] = [rstd*w, m*s - b]  (f32r: feeds the corr matmul)
                sb = small.tile([P, 2], F32R, tag="sb", name="sb")
                nc.vector.tensor_scalar_mul(sb[:, 0:1], gp[:, 4 + 2 * cb : 5 + 2 * cb], gnp(rb, ln, "w", cb))
                nc.vector.tensor_tensor(out=sb[:, 1:2], in0=gp[:, 5 + 2 * cb : 6 + 2 * cb], in1=_f(sb[:, 0:1]), op=OP.mult)
                nc.vector.tensor_scalar_sub(sb[:, 1:2], _f(sb[:, 1:2]), gnp(rb, ln, "b", cb))
                sbs.append(sb)
                # conv pad columns hold -b2 (the stored value of a zero activation)
                nc.vector.tensor_copy(out=dst[cb][:, 0:1], in_=sb[:, 1:2])
                nc.vector.tensor_copy(out=dst[cb][:, L + 1 : L + 2], in_=sb[:, 1:2])
            # corr[oc] = sum_ic Wsum[ic, oc] * b2[ic], directly as per-ocb
            # columns: lhsT = Wsum block, rhs = -b2 column, negated in the
            # PSUM->SBUF copy
            j = (0 if rb == "r1" else 2) + (ln - 1)
            for ocb in range(CB):
                for icb in range(CB):
                    # moving operand is the whole [s, -b2] sb tile (f32r needs
                    # even offset/width); column 0 of the product is unused
                    nc.tensor.matmul(
                        gp[:, 8 + 2 * ocb : 10 + 2 * ocb],
                        wsum_sb[:, icb, j, ocb * P : (ocb + 1) * P],
                        sbs[icb][:],
                        start=(icb == 0), stop=(icb == 1),
                    )
            corr = persl.tile([P, 2], F32, tag="corr", name="corr")
            for ocb in range(CB):
                nc.vector.tensor_scalar_mul(corr[:, ocb : ocb + 1], gp[:, 9 + 2 * ocb : 10 + 2 * ocb], -1.0)
            # apply on DVE in NL chunks, cb-interleaved so the first conv
            # group (which reads both cb blocks) unblocks earliest
            for i in range(NL):
                for cb in range(CB):
                    nc.vector.tensor_scalar(
                        dst[cb][:, 1 + i * LS : 1 + (i + 1) * LS],
                        _f(src[cb][:, 1 + i * LS : 1 + (i + 1) * LS]),
                        _f(sbs[cb][:, 0:1]),
                        _f(sbs[cb][:, 1:2]),
                        OP.mult,
                        OP.max,
                    )
            return corr

        def conv3(src, wt, consume, nalt=0):
            """3-tap conv over padded f32r src; consume(ocb, ls, psum_ap).

            The first `nalt` output groups draw their PSUM from the psc ring
            instead of pacc — after an attention phase, pacc's slots are still
            pinned by the softmax tail (psav reads), while psc's are free."""
            gi = 0
            for ls in range(NL):
                for ocb in range(CB):
                    if gi < nalt:
                        ps = psc.tile([P, 2, LS], F32, tag="sc", name="acc_alt")[:, 0, :]
                    else:
                        ps = pacc.tile([P, LS], F32, tag="acc", name="acc")[:]
                    gi += 1
                    k = 0
                    for icb in range(CB):
                        for tap in range(3):
                            nc.tensor.matmul(
                                ps,
                                wt[:, icb, tap, ocb * P : (ocb + 1) * P],
                                src[icb][:, ls * LS + tap : ls * LS + tap + LS],
                                start=(k == 0),
                                stop=(k == 5),
                            )
                            k += 1
                    consume(ocb, ls, ps)

        # ------- per-sample state + fine-grained stage closures -------
        def make_sample(s):
            st = {}

            def load():
                # x/t ride the ScalarE HWDGE queue (fast descriptor issue; the
                # gpsimd SWDGE queue pays ~0.6us of software descriptor gen
                # per transfer and carries the consts instead).  Do NOT route
                # x through sync: its queue stalls multi-us between DMAs
                # (measured twice, ~55us end-to-end).  s0 is issued first so
                # its first groupnorm unblocks earliest.
                with nc.named_scope(f"s{s}_load"):
                    st["xp"] = alloc_padded("pad", padp)
                    for cb in range(CB):
                        for i in range(NL):
                            nc.scalar.dma_start(
                                st["xp"][cb][:, 1 + i * LS : 1 + (i + 1) * LS],
                                x_d[s, cb * P : (cb + 1) * P, i * LS : (i + 1) * LS],
                            )
                    st["t2"] = t2p.tile([P, CB, 2], F32, tag="t2", name="t2")
                    nc.scalar.dma_start(st["t2"][:], t_d[s])

            def gn1(rb, srckey, dstkey):
                def f():
                    a = alloc_padded("act", actp, BF16)
                    st[dstkey] = a
                    with nc.named_scope(f"s{s}_{rb}_gn1"):
                        st[f"corr_{rb}1"] = gn_relu(st[srckey], a, rb, 1)
                return f

            def conv1(rb, rbi, akey, hkey):
                def f():
                    h = alloc_padded("pad", padp)
                    st[hkey] = h
                    t2 = st["t2"]
                    corr = st[f"corr_{rb}1"]
                    with nc.named_scope(f"s{s}_{rb}_conv1"):
                        # fold the gn-bias conv correction into the t vector
                        tadj = persl.tile([P, 2], F32, tag="tadj", name="tadj")
                        nc.vector.tensor_tensor(out=tadj[:], in0=t2[:, :, rbi : rbi + 1], in1=corr[:], op=OP.add)

                        def eat1(ocb, ls, ps):
                            nc.vector.tensor_scalar_add(
                                h[ocb][:, 1 + ls * LS : 1 + (ls + 1) * LS], ps,
                                tadj[:, ocb : ocb + 1],
                            )
                        conv3(st[akey], w1_sb[rb], eat1, nalt=3 if rb == "r2" else 0)
                return f

            def gn2(rb, hkey, dstkey):
                def f():
                    a2 = alloc_padded("act", actp, BF16)
                    st[dstkey] = a2
                    with nc.named_scope(f"s{s}_{rb}_gn2"):
                        st[f"corr_{rb}2"] = gn_relu(st[hkey], a2, rb, 2)
                return f

            def conv2(rb, srckey, a2key, final):
                def f():
                    src = st[srckey]
                    corr = st[f"corr_{rb}2"]
                    res = None
                    if not final:
                        res = [seqp.tile([P, L], BF16, tag=f"res{cb}", name=f"res{cb}") for cb in range(CB)]
                        st["x1"] = res
                    with nc.named_scope(f"s{s}_{rb}_conv2"):
                        def eat2(ocb, ls, ps):
                            if rb in c2b_sb:
                                nc.vector.tensor_scalar_add(ps, ps, c2b_sb[rb][:, ocb : ocb + 1])
                            resid = _f(src[ocb][:, 1 + ls * LS : 1 + (ls + 1) * LS])
                            ccol = corr[:, ocb : ocb + 1]
                            if final:
                                ot = outp.tile([P, LS], F32, tag="out", name="ot")
                                nc.vector.scalar_tensor_tensor(
                                    out=ot[:], in0=ps, scalar=ccol, in1=resid,
                                    op0=OP.add, op1=OP.add,
                                )
                                nc.sync.dma_start(
                                    out_d[s, ocb * P : (ocb + 1) * P, ls * LS : (ls + 1) * LS], ot[:]
                                )
                            else:
                                nc.vector.scalar_tensor_tensor(
                                    out=res[ocb][:, ls * LS : (ls + 1) * LS],
                                    in0=ps, scalar=ccol, in1=resid,
                                    op0=OP.add, op1=OP.add,
                                )
                        conv3(st[a2key], w2_sb[rb], eat2)
                return f

            def kqv():
                x1 = st["x1"]
                kt = seqp.tile([P, CB, L], FP8, tag="kt", name="kt")
                qt = seqp.tile([P, CB, L], FP8, tag="qt", name="qt")
                vt = vtp.tile([P, NP, 2, C], FP8, tag="vt", name="vt")
                st["kt"], st["qt"], st["vt"] = kt, qt, vt
                with nc.named_scope(f"s{s}_kqv"):
                    cp = 0
                    for j, dst in ((0, kt), (1, qt)):
                        for ocb in range(CB):
                            off = j * C + ocb * P
                            for ls in range(NL):
                                if cp < 5:
                                    # dodge the attention-tail psav ring wait
                                    ps = psc.tile([P, 2, LS], F32, tag="sc", name="acc_alt")[:, 0, :]
                                else:
                                    ps = pacc.tile([P, LS], F32, tag="acc", name="acc")[:]
                                for icb in range(CB):
                                    nc.tensor.matmul(
                                        ps,
                                        wkqv_sb[:, icb, off : off + P],
                                        x1[icb][:, ls * LS : (ls + 1) * LS],
                                        start=(icb == 0),
                                        stop=(icb == 1),
                                    )
                                dsl = dst[:, ocb, ls * LS : (ls + 1) * LS]
                                cp += 1
                                # kqv drains ride ScalarE (idle during this
                                # phase; DVE is saturated by the conv eats +
                                # bn_stats running concurrently)
                                if linb_sb is not None:
                                    nc.scalar.activation(
                                        out=dsl, in_=ps, func=AF.Identity,
                                        bias=linb_sb[:, j * CB + ocb : j * CB + ocb + 1],
                                    )
                                else:
                                    nc.scalar.activation(out=dsl, in_=ps, func=AF.Copy)
                    # vT[l, c] (l on partitions) for the attention output matmul
                    for lb in range(KB):
                        ps = pacc.tile([P, LS], F32, tag="acc", name="acc")
                        for icb in range(CB):
                            nc.tensor.matmul(
                                ps[:, :C],
                                x1[icb][:, lb * P : (lb + 1) * P],
                                wkqv_sb[:, icb, 2 * C : 3 * C],
                                start=(icb == 0),
                                stop=(icb == 1),
                            )
                        # v bias (if any) is added to av after softmax: sum(a)=1
                        nc.scalar.activation(out=vt[:, lb // 2, lb % 2, :], in_=ps[:, :C], func=AF.Copy)

            def attn():
                kt, qt, vt = st["kt"], st["qt"], st["vt"]
                av = alloc_padded("pad", padp)
                st["av"] = av
                for qs in range(NL):
                    with nc.named_scope(f"s{s}_attn{qs}"):
                        dn = paux.tile([P, LS], F32, tag="aux", name="dn")
                        psav = [pacc.tile([P, LS], F32, tag="acc", name="psav") for _ in range(CB)]
                        for p in range(NP):
                            ex = expp.tile([P, 2, LS], FP8, tag="exp", name="exp")
                            sc = psc.tile([P, 2, LS], F32, tag="sc", name="sc")
                            for i in range(2):
                                kbg = 2 * p + i
                                nc.tensor.matmul(
                                    sc[:, i, :],
                                    kt[:, :, kbg * P : (kbg + 1) * P],
                                    qt[:, :, qs * LS : (qs + 1) * LS],
                                    start=True, stop=True, perf_mode=DR,
                                )
                            # one 1024-col exp per k-block pair
                            nc.scalar.activation(
                                out=ex[:, :, :], in_=sc[:, :, :], func=AF.Exp,
                                bias=shift_sb[:], scale=SCALE,
                            )
                            nc.tensor.matmul(
                                dn[0:1, :], ones8_sb[:, :, 0:1], ex[:],
                                start=(p == 0), stop=(p == NP - 1), perf_mode=DR,
                            )
                            for cb in range(CB):
                                nc.tensor.matmul(
                                    psav[cb][:],
                                    vt[:, p, :, cb * P : (cb + 1) * P],
                                    ex[:],
                                    start=(p == 0),
                                    stop=(p == NP - 1),
                                    perf_mode=DR,
                                )
                        lnd = rdsp.tile([1, LS], F32, tag="lnd", name="lnd")
                        nc.scalar.activation(out=lnd[:], in_=dn[0:1, :], func=AF.Ln)
                        rd = rdsp.tile([1, LS], F32R, tag="rd", name="rd")
                        nc.scalar.activation(out=rd[:], in_=lnd[:], func=AF.Exp, scale=-1.0)
                        # broadcast 1/denom across partitions via K=1 ones-matmul
                        # OVERWRITING the dn bank (dn is already consumed by the
                        # Ln): one paux alloc per qs keeps the aux ring free for
                        # the overlapped sample's groupnorm aggregation scratch
                        nc.tensor.matmul(dn[:], onesr_sb[:], rd[:], start=True, stop=True)
                        rdb = rdbp.tile([P, LS], F32, tag="rdbs", name="rdb")
                        nc.scalar.activation(out=rdb[:], in_=dn[:], func=AF.Copy)
                        for cb in range(CB):
                            avs = av[cb][:, 1 + qs * LS : 1 + (qs + 1) * LS]
                            nc.vector.tensor_tensor(out=avs, in0=psav[cb][:], in1=rdb[:], op=OP.mult)
                            if linb_sb is not None:
                                nc.vector.tensor_scalar_add(
                                    avs, _f(avs), linb_sb[:, 2 * CB + cb : 2 * CB + cb + 1]
                                )

            return {
                "load": load,
                "gn1": gn1("r1", "xp", "a"),
                "conv1": conv1("r1", 0, "a", "h"),
                "gn2": gn2("r1", "h", "a2"),
                "conv2": conv2("r1", "xp", "a2", final=False),
                "kqv": kqv,
                "attn": attn,
                "rgn1": gn1("r2", "av", "ra"),
                "rconv1": conv1("r2", 1, "ra", "rh"),
                "rgn2": gn2("r2", "rh", "ra2"),
                "rconv2": conv2("r2", "av", "ra2", final=True),
            }

        ph = [make_sample(s) for s in range(samples)]
        warmup_pe()
        if samples == 2:
            s0, s1 = ph
            # interleave the two samples so every groupnorm stat chain and
            # softmax tail overlaps the other sample's matmuls
            s0["load"](); s1["load"]()
            load_late_consts()
            s0["gn1"](); s1["gn1"]()
            s0["conv1"](); s0["gn2"]()
            s1["conv1"](); s1["gn2"]()
            s0["conv2"]()
            s1["conv2"]()
            s0["kqv"](); s0["attn"]()
            s1["kqv"]()
            s0["rgn1"]()
            s1["attn"]()
            s0["rconv1"]()
            s1["rgn1"]()
            s0["rgn2"]()
            s1["rconv1"]()
            s1["rgn2"]()
            s0["rconv2"]()
            s1["rconv2"]()
        else:
            for pi, p_ in enumerate(ph):
                p_["load"]()
                if pi == 0:
                    load_late_consts()
                for k in ("gn1", "conv1", "gn2", "conv2", "kqv", "attn",
                          "rgn1", "rconv1", "rgn2", "rconv2"):
                    p_[k]()

    nc.finalize()
    return nc


def _pack_conv_w(w):
    """(O, I, 3) -> [P, icb, tap, oc] bf16."""
    w = np.asarray(w, dtype=np.float32)
    o, i, k = w.shape
    return np.ascontiguousarray(
        w.transpose(1, 2, 0).reshape(CB, P, 3, o).transpose(1, 0, 2, 3)
    ).astype(ml_dtypes.bfloat16)


def _pack_gn(v):
    """(256,) -> [P, CB]"""
    return np.ascontiguousarray(np.asarray(v, dtype=np.float32).reshape(CB, P).T)


def make_in_maps(inp, use_bias):
    """Host-side packing; returns the per-core input maps."""
    gind = np.zeros((P, GPB), np.float32)
    bind = np.zeros((CB, P, P), np.float32)
    for cc in range(P):
        gind[cc, cc // 8] = 0.125
        for cb in range(CB):
            bind[cb, cb * 32 + cc // 8, cc] = 1.0
    shared = {
        "wkqvt": np.ascontiguousarray(
            inp["lin_w"][:, :, 0].T.reshape(CB, P, 3 * C).transpose(1, 0, 2)
        ).astype(ml_dtypes.bfloat16),
        "gind": gind,
        "bind": bind,
        "ones8": np.ones((P, CB, 16), ml_dtypes.float8_e4m3),
        "onesr": np.ones((1, P), np.float32),
    }
    gnall = np.empty((P, CB, 8), np.float32)
    wsum = np.empty((P, CB, 4, C), np.float32)
    for rbi, rb in enumerate(("r1", "r2")):
        shared[f"{rb}_w1t"] = _pack_conv_w(inp[f"{rb}_c1_w"])
        shared[f"{rb}_w2t"] = _pack_conv_w(inp[f"{rb}_c2_w"])
        for ln in (1, 2):
            gnall[:, :, rbi * 4 + (ln - 1) * 2 + 0] = _pack_gn(inp[f"{rb}_gn{ln}_w"])
            gnall[:, :, rbi * 4 + (ln - 1) * 2 + 1] = _pack_gn(inp[f"{rb}_gn{ln}_b"])
            # wsum[icp, icb, j, oc] = sum_tap w[oc, ic, tap]
            w = np.asarray(inp[f"{rb}_c{ln}_w"], np.float32).sum(-1)  # (O, I)
            wsum[:, :, rbi * 2 + (ln - 1), :] = w.T.reshape(CB, P, C).transpose(1, 0, 2)
    shared["gnallt"] = gnall
    shared["wsumt"] = wsum
    if "c2b_r1" in use_bias:
        shared["r1_c2bs"] = _pack_gn(inp["r1_c2_b"])
    if "c2b_r2" in use_bias:
        shared["r2_c2bs"] = _pack_gn(inp["r2_c2_b"])
    if "linb" in use_bias:
        shared["lin_bs"] = np.ascontiguousarray(inp["lin_b"].reshape(3 * CB, P).T)

    # per-sample conv1 bias vector: t[s] + c1_b per res block -> [P, CB, 2]
    tfull = inp["t"][:, :, 0]  # (B, C)
    nb = inp["x"].shape[0]
    tv = np.empty((nb, P, CB, 2), np.float32)
    for rbi, rb in enumerate(("r1", "r2")):
        v = tfull + inp[f"{rb}_c1_b"][None, :]
        tv[:, :, :, rbi] = v.reshape(nb, CB, P).transpose(0, 2, 1)

    in_maps = []
    for c in range(NCORES):
        sl = slice(S * c, S * (c + 1))
        m = dict(shared)
        m["x"] = inp["x"][sl]
        m["tv"] = np.ascontiguousarray(tv[sl])
        in_maps.append(m)
    return in_maps


_CACHE = {}


def kernel(**inputs):
    inp = {k: np.ascontiguousarray(np.asarray(v, dtype=np.float32)) for k, v in inputs.items()}

    use_bias = []
    if np.any(inp["r1_c2_b"]):
        use_bias.append("c2b_r1")
    if np.any(inp["r2_c2_b"]):
        use_bias.append("c2b_r2")
    if np.any(inp["lin_b"]):
        use_bias.append("linb")
    use_bias = tuple(use_bias)

    if ("nc", use_bias) not in _CACHE:
        _CACHE[("nc", use_bias)] = build_program(S, use_bias)
    nc = _CACHE[("nc", use_bias)]

    in_maps = make_in_maps(inp, use_bias)
    res = _bu.run_bass_kernel_spmd(nc, in_maps, core_ids=list(range(NCORES)))
    out = np.concatenate([res.results[c]["out"] for c in range(NCORES)], axis=0)
    return out.astype(np.float32)



# revision 36
# speedup vs baseline: 1.0252x; 1.0252x over previous
"""Trainium2 Bass kernel for nn_MidAttnBlock (res-block -> full LxL attention -> res-block).

Contract: kernel(**inputs) takes the FULL inputs of reference.setup_inputs()
(x: (16,256,2048) f32, t: (16,256,1) f32, plus conv/groupnorm/linear params)
and returns the FULL (16,256,2048) f32 output.  Data-parallel over batch on
8 NeuronCores, 2 samples per core; each core runs an identical Bass program.

Convs and the kqv projection run in bf16 (full-rate PE, half the SBUF/HBM
of f32r).  The attention (scores, softmax weights, A@V) runs in fp8e4 with
DoubleRow matmuls (256-deep contraction per PE pass).  ~7.5e-3 end-to-end
relative error (threshold 2e-2).  exp is computed as exp(s/16 - 4) so the
softmax weights fit fp8e4's range; the shift cancels in the normalization.

The groupnorm relu applies run on the Vector engine as one
tensor_scalar((x*s) max -b2) per chunk: the activation tiles store
relu(gn(x)) - b2 with pad columns = -b2, and the following conv adds back
the per-output-channel constant corr = Wsum @ b2 (folded into the t-vector
add for conv1 and the residual add for conv2).  This keeps ScalarE free for
the attention exps and the kqv PSUM drains.

The two samples on each core are issued interleaved
(s0:r1,kqv | s1:r1 | s0:attn | s1:kqv | s0:r2 | s1:attn | s1:r2) so the
groupnorm stat chains and softmax tails of one sample overlap the other
sample's matmuls.

Self-contained: all shapes/sharding hardcoded.
"""

import json as _json

import ml_dtypes
import numpy as np

import concourse.bass as bass
import concourse.bass2jax as _b2j
import concourse.bass_utils as _bu
import concourse.tile as tile
from concourse import mybir
from concourse.vector_clock import ScopedClock, VectorClock


def _split_bir_waits(bir_json):
    """The walrus_driver in this container encodes at most ONE sync-wait per
    instruction (and none on Drain).  Tile's sem assigner attaches several.
    Rewrite the BIR: excess waits move to single-wait NoOps inserted directly
    before the instruction on the same engine."""
    m = _json.loads(bir_json)
    ctr = 0
    for fn in m.get("functions", []):
        for bb in fn.get("blocks", []):
            out = []
            for ins in bb.get("instructions", []):
                si = ins.get("sync_info")
                waits = (si or {}).get("on_wait") or []
                keep = 0 if ins.get("opcode") == "Drain" else 1
                if len(waits) > keep:
                    nmove = len(waits) - keep
                    for w in waits[:nmove]:
                        ctr += 1
                        out.append({
                            "debug": ins.get("debug", 0),
                            "engine": ins["engine"],
                            "ins": [],
                            "name": f"{ins['name']}-wsp{ctr}",
                            "opcode": "NoOp",
                            "outs": [],
                            "sync_info": {"on_update": [], "on_wait": [w]},
                        })
                    si["on_wait"] = waits[nmove:]
                out.append(ins)
            bb["instructions"] = out
    return _json.dumps(m).encode()


_orig_compile_bir_kernel = _bu.compile_bir_kernel


def _compile_bir_splitwaits(bir_json, tmpdir, neff_name="file.neff"):
    return _orig_compile_bir_kernel(_split_bir_waits(bir_json), tmpdir, neff_name)


if getattr(_bu.compile_bir_kernel, "__name__", "") != "_compile_bir_splitwaits":
    _bu.compile_bir_kernel = _compile_bir_splitwaits
    _b2j.compile_bir_kernel = _compile_bir_splitwaits


F32 = mybir.dt.float32
F32R = mybir.dt.float32r
BF16 = mybir.dt.bfloat16
FP8 = mybir.dt.float8e4
AF = mybir.ActivationFunctionType
OP = mybir.AluOpType
DR = mybir.MatmulPerfMode.DoubleRow

P = 128          # partitions
C = 256          # channels
CB = 2           # channel blocks of 128
L = 2048         # sequence length
LS = 512         # l-slice (matmul moving dim)
NL = L // LS     # 4 slices
KB = L // P      # 16 k-blocks for attention
NP = KB // 2     # 8 k-block pairs (DoubleRow)
GPB = 16         # groups per channel-block (32 groups, 8 ch each)
EPS = 1e-5
S = 2            # samples per core
NCORES = 8
SCALE = 1.0 / 16.0   # 1/sqrt(C)
ESHIFT = -4.0        # exp(s*SCALE + ESHIFT): keeps softmax weights in fp8e4 range


class _TileContextPatched(tile.TileContext):
    """TileContext whose kernel-tail drain carries no sem waits (the container
    walrus rejects waits on Drain); one SP NOP per proc carries them instead."""

    def _drain_and_barrier(self, tick_clock, wait_clock):
        gc = tick_clock.global_clock
        n = len(gc)
        for p in range(n):
            v = gc[p]
            if v > 0:
                vec = [0] * n
                vec[p] = v
                nop = self.nc.sync.nop()
                wait_clock.add_sem_waits(nop.ins, ScopedClock({None: VectorClock(vec)}))
        self.nc.sync.drain()
        self.nc.all_engine_barrier()
        assert self.sems is not None
        popped = self.nc._tile_sem_poison_stack.pop()
        assert popped is self._sem_poison
        self.nc.clear_and_free_semaphores(list(self.sems.allocated().values()))
        self.nc.all_engine_barrier()


def _f(ap):
    """Read an f32r tile as plain f32 (same bits) for VectorE/ScalarE inputs."""
    return ap.bitcast(F32)


def build_program(samples=S, use_bias=()):
    """Build the per-core Bass program (identical on all cores).

    use_bias: subset of {"c2b_r1", "c2b_r2", "linb"} enabling extra adds for
    biases that setup_inputs() keeps at zero.
    """
    nc = bass.Bass()

    # ---- DRAM I/O (per core) ----
    x_d = nc.dram_tensor("x", (samples, C, L), F32R, kind="ExternalInput")
    # t + conv1 bias, host-packed [samples, P, CB, 2(resblock)]
    t_d = nc.dram_tensor("tv", (samples, P, CB, 2), F32, kind="ExternalInput")
    w_conv = {}
    for rb in ("r1", "r2"):
        # host-packed [P(ic within block), icb, tap, oc]
        w_conv[rb, 1] = nc.dram_tensor(f"{rb}_w1t", (P, CB, 3, C), BF16, kind="ExternalInput")
        w_conv[rb, 2] = nc.dram_tensor(f"{rb}_w2t", (P, CB, 3, C), BF16, kind="ExternalInput")
    wkqv_d = nc.dram_tensor("wkqvt", (P, CB, 3 * C), BF16, kind="ExternalInput")
    # tap-summed conv weights for the groupnorm-bias correction:
    # wsum[icp, icb, j, oc] = sum_tap w_j[oc, ic, tap], j in (r1c1, r1c2, r2c1, r2c2)
    wsum_d = nc.dram_tensor("wsumt", (P, CB, 4, C), F32R, kind="ExternalInput")
    # all 8 groupnorm affine vectors in one tensor: [P, CB, rb*4+(ln-1)*2+wb]
    gnall_d = nc.dram_tensor("gnallt", (P, CB, 8), F32, kind="ExternalInput")
    c2b_d = {}
    if "c2b_r1" in use_bias:
        c2b_d["r1"] = nc.dram_tensor("r1_c2bs", (P, CB), F32, kind="ExternalInput")
    if "c2b_r2" in use_bias:
        c2b_d["r2"] = nc.dram_tensor("r2_c2bs", (P, CB), F32, kind="ExternalInput")
    linb_d = None
    if "linb" in use_bias:
        linb_d = nc.dram_tensor("lin_bs", (P, 3 * CB), F32, kind="ExternalInput")
    gind_d = nc.dram_tensor("gind", (P, GPB), F32R, kind="ExternalInput")  # 1/8 group indicator
    bind_d = nc.dram_tensor("bind", (CB, P, P), F32R, kind="ExternalInput")    # group->channel broadcast
    ones8_d = nc.dram_tensor("ones8", (P, CB, 16), FP8, kind="ExternalInput")
    onesr_d = nc.dram_tensor("onesr", (1, P), F32R, kind="ExternalInput")
    out_d = nc.dram_tensor("out", (samples, C, L), F32, kind="ExternalOutput")

    with _TileContextPatched(nc) as tc, \
         tc.tile_pool(name="consts", bufs=1) as consts, \
         tc.tile_pool(name="padp", bufs=4) as padp, \
         tc.tile_pool(name="actp", bufs=2) as actp, \
         tc.tile_pool(name="seqp", bufs=2) as seqp, \
         tc.tile_pool(name="vtp", bufs=1) as vtp, \
         tc.tile_pool(name="expp", bufs=3) as expp, \
         tc.tile_pool(name="outp", bufs=2) as outp, \
         tc.tile_pool(name="rdbp", bufs=2) as rdbp, \
         tc.tile_pool(name="rdsp", bufs=2) as rdsp, \
         tc.tile_pool(name="small", bufs=4) as small, \
         tc.tile_pool(name="persl", bufs=12) as persl, \
         tc.tile_pool(name="t2p", bufs=2) as t2p, \
         tc.tile_pool(name="pacc", bufs=3, space="PSUM") as pacc, \
         tc.tile_pool(name="psc", bufs=2, space="PSUM") as psc, \
         tc.tile_pool(name="paux", bufs=1, space="PSUM") as paux:

        # ---- persistent constants / weights in SBUF ----
        # All const loads ride the gpsimd SWDGE queue (descriptor gen is
        # ~0.6us each, so the count is kept low and ordered so the tensors
        # gating the pipeline head land first); x/t ride the fast ScalarE
        # HWDGE queue concurrently.
        gind_sb = consts.tile([P, GPB], F32R, tag="gind", name="gind")
        gnall_sb = consts.tile([P, CB, 8], F32, tag="gnall", name="gnall")
        wsum_sb = consts.tile([P, CB, 4, C], F32R, tag="wsum", name="wsum")
        bind_sb = consts.tile([P, CB, P], F32R, tag="bind", name="bind")

        def load_early_consts():
            # issued between the s0 and s1 x loads: everything the s0
            # groupnorm chain + first conv needs, in dependency order
            nc.gpsimd.dma_start(gind_sb[:], gind_d[:])
            nc.gpsimd.dma_start(gnall_sb[:], gnall_d[:])
            nc.gpsimd.dma_start(wsum_sb[:, :, 0:2, :], wsum_d[:, :, 0:2, :])
            nc.gpsimd.dma_start(bind_sb[:], bind_d.rearrange("cb p c -> p cb c"))
            nc.gpsimd.dma_start(w1_sb["r1"][:], w_conv["r1", 1][:])

        def gnp(rb, ln, wb, cb):
            idx = (0 if rb == "r1" else 4) + (ln - 1) * 2 + (0 if wb == "w" else 1)
            return gnall_sb[:, cb, idx : idx + 1]

        w1_sb = {}
        w2_sb = {}
        late_consts = []
        for rb in ("r1", "r2"):
            w1_sb[rb] = consts.tile([P, CB, 3, C], BF16, tag=f"w1_{rb}", name=f"w1_{rb}")
            w2_sb[rb] = consts.tile([P, CB, 3, C], BF16, tag=f"w2_{rb}", name=f"w2_{rb}")
            if rb == "r1":
                late_consts.insert(0, (w2_sb[rb][:], w_conv[rb, 2][:]))
            else:
                late_consts.append((w1_sb[rb][:], w_conv[rb, 1][:]))
                late_consts.append((w2_sb[rb][:], w_conv[rb, 2][:]))
        wkqv_sb = consts.tile([P, CB, 3 * C], BF16, tag="wkqv", name="wkqv")
        late_consts.insert(1, (wkqv_sb[:], wkqv_d[:]))
        late_consts.append((wsum_sb[:, :, 2:4, :], wsum_d[:, :, 2:4, :]))
        c2b_sb = {}
        for rb, d in c2b_d.items():
            c2b_sb[rb] = consts.tile([P, CB], F32, tag=f"c2b_{rb}", name=f"c2b_{rb}")
            late_consts.append((c2b_sb[rb][:], d[:]))
        linb_sb = None
        if linb_d is not None:
            linb_sb = consts.tile([P, 3 * CB], F32, tag="linb", name="linb")
            late_consts.append((linb_sb[:], linb_d[:]))
        ones8_sb = consts.tile([P, CB, 16], FP8, tag="ones8", name="ones8")
        late_consts.append((ones8_sb[:], ones8_d[:]))
        onesr_sb = consts.tile([1, P], F32R, tag="onesr", name="onesr")
        late_consts.append((onesr_sb[:], onesr_d[:]))

        def load_late_consts():
            # issued after the x loads so they don't contend for HBM bandwidth
            # ahead of the first groupnorm
            for ap, src in late_consts:
                nc.gpsimd.dma_start(ap, src)
        eps_sb = consts.tile([P, 1], F32, tag="eps", name="eps")
        nc.vector.memset(eps_sb[:], EPS)
        shift_sb = consts.tile([P, 1], F32, tag="shift", name="shift")
        nc.vector.memset(shift_sb[:], ESHIFT)
        zero2 = consts.tile([P, 2], F32, tag="zero2", name="zero2")
        nc.vector.memset(zero2[:], 0.0)

        def warmup_pe(xt):
            # ~3us of tiny back-to-back matmuls (on the first-loaded x chunk,
            # values irrelevant) while the rest of the DMAs land, so the PE
            # p-state is at full clock when the first conv group issues
            wk = paux.tile([P, LS], F32, tag="aux", name="warm")
            for _ in range(96):
                nc.tensor.matmul(wk[:GPB, 0:GPB], xt[:, 1:17], xt[:, 1:17], start=True, stop=True)

        def alloc_padded(tag, pool, dt=F32R):
            """[P, L+2] tile per channel block; data cols [1, L+1).

            Edge columns are only meaningful for conv sources (the gn_relu
            destinations), where gn_relu writes them to -b2."""
            return [
                pool.tile([P, L + 2], dt, tag=f"{tag}{cb}", name=f"{tag}{cb}")
                for cb in range(CB)
            ]

        def gn_relu(src, dst, rb, ln):
            """dst = relu(groupnorm(src)*w + b) - b2, with pad cols = -b2.

            b2 is the effective per-channel bias (b - m*rstd*w); subtracting it
            turns the ScalarE relu into one DVE tensor_scalar (x*s max -b2),
            and padding with -b2 keeps the following conv exact up to a
            per-output-channel constant corr = Wsum @ b2 that the conv
            consumer adds back (returned here as a [P, CB] column tile)."""
            gp = paux.tile([P, LS], F32, tag="aux", name="gbc")
            for cb in range(CB):
                stats = small.tile([P, NL, 6], F32, tag="stats", name="stats")
                for i in range(NL):
                    nc.vector.bn_stats(out=stats[:, i, :], in_=_f(src[cb][:, 1 + i * LS : 1 + (i + 1) * LS]))
                mv = small.tile([P, 2], F32, tag="mv", name="mv")
                nc.vector.bn_aggr(out=mv[:], in_=stats[:])
                # tmp = [mean_c, E[x^2]_c]  (f32r: feeds the aggregation matmul)
                tmp = small.tile([P, 2], F32R, tag="tmp", name="tmp")
                nc.vector.tensor_copy(out=tmp[:, 0:1], in_=mv[:, 0:1])
                nc.vector.tensor_tensor(out=tmp[:, 1:2], in0=mv[:, 0:1], in1=mv[:, 0:1], op=OP.mult)
                nc.vector.tensor_tensor(out=tmp[:, 1:2], in0=_f(tmp[:, 1:2]), in1=mv[:, 1:2], op=OP.add)
                nc.tensor.matmul(gp[:GPB, 2 * cb : 2 * cb + 2], gind_sb[:], tmp[:], start=True, stop=True)
            # merged group stats; block-cb groups live at partition offset 32*cb
            NG = 32 * CB
            gs = small.tile([NG, 2], F32, tag="gs", name="gs")
            nc.vector.tensor_copy(out=gs[:], in_=zero2[:NG])
            for cb in range(CB):
                nc.vector.tensor_copy(out=gs[cb * 32 : cb * 32 + GPB, :], in_=gp[:GPB, 2 * cb : 2 * cb + 2])
            var = small.tile([NG, 1], F32, tag="var", name="var")
            nc.vector.tensor_tensor(out=var[:], in0=gs[:, 0:1], in1=gs[:, 0:1], op=OP.mult)
            nc.vector.tensor_tensor(out=var[:], in0=gs[:, 1:2], in1=var[:], op=OP.subtract)
            nc.scalar.activation(out=var[:], in_=var[:], func=AF.Ln, bias=eps_sb[:NG])
            rstd = small.tile([NG, 1], F32, tag="rstd", name="rstd")
            nc.scalar.activation(out=rstd[:], in_=var[:], func=AF.Exp, scale=-0.5)
            # pack [rstd_g, +m_g], zero-extended to 128 partitions
            gpk = small.tile([P, 2], F32R, tag="gpk", name="gpk")
            nc.vector.tensor_copy(out=gpk[:], in_=zero2[:])
            nc.vector.tensor_copy(out=gpk[:NG, 0:1], in_=rstd[:])
            nc.vector.tensor_copy(out=gpk[:NG, 1:2], in_=gs[:, 0:1])
            sbs = []
            for cb in range(CB):
                # broadcast to channels: bc[c, :] = [rstd_g(c), m_g(c)]
                nc.tensor.matmul(gp[:, 4 + 2 * cb : 6 + 2 * cb], bind_sb[:, cb, :], gpk[:], start=True, stop=True)
                # sb = [s, -b2] = [rstd*w, m*s - b]  (f32r: feeds the corr matmul)
                sb = small.tile([P, 2], F32R, tag="sb", name="sb")
                nc.vector.tensor_scalar_mul(sb[:, 0:1], gp[:, 4 + 2 * cb : 5 + 2 * cb], gnp(rb, ln, "w", cb))
                nc.vector.tensor_tensor(out=sb[:, 1:2], in0=gp[:, 5 + 2 * cb : 6 + 2 * cb], in1=_f(sb[:, 0:1]), op=OP.mult)
                nc.vector.tensor_scalar_sub(sb[:, 1:2], _f(sb[:, 1:2]), gnp(rb, ln, "b", cb))
                sbs.append(sb)
                # conv pad columns hold -b2 (the stored value of a zero activation)
                nc.vector.tensor_copy(out=dst[cb][:, 0:1], in_=sb[:, 1:2])
                nc.vector.tensor_copy(out=dst[cb][:, L + 1 : L + 2], in_=sb[:, 1:2])
            # corr[oc] = sum_ic Wsum[ic, oc] * b2[ic], directly as per-ocb
            # columns: lhsT = Wsum block, rhs = -b2 column, negated in the
            # PSUM->SBUF copy
            j = (0 if rb == "r1" else 2) + (ln - 1)
            for ocb in range(CB):
                for icb in range(CB):
                    # moving operand is the whole [s, -b2] sb tile (f32r needs
                    # even offset/width); column 0 of the product is unused
                    nc.tensor.matmul(
                        gp[:, 8 + 2 * ocb : 10 + 2 * ocb],
                        wsum_sb[:, icb, j, ocb * P : (ocb + 1) * P],
                        sbs[icb][:],
                        start=(icb == 0), stop=(icb == 1),
                    )
            corr = persl.tile([P, 2], F32, tag="corr", name="corr")
            for ocb in range(CB):
                nc.vector.tensor_scalar_mul(corr[:, ocb : ocb + 1], gp[:, 9 + 2 * ocb : 10 + 2 * ocb], -1.0)
            # apply on DVE in NL chunks, cb-interleaved so the first conv
            # group (which reads both cb blocks) unblocks earliest
            for i in range(NL):
                for cb in range(CB):
                    nc.vector.tensor_scalar(
                        dst[cb][:, 1 + i * LS : 1 + (i + 1) * LS],
                        _f(src[cb][:, 1 + i * LS : 1 + (i + 1) * LS]),
                        _f(sbs[cb][:, 0:1]),
                        _f(sbs[cb][:, 1:2]),
                        OP.mult,
                        OP.max,
                    )
            return corr

        def conv3(src, wt, consume, nalt=0):
            """3-tap conv over padded f32r src; consume(ocb, ls, psum_ap).

            The first `nalt` output groups draw their PSUM from the psc ring
            instead of pacc — after an attention phase, pacc's slots are still
            pinned by the softmax tail (psav reads), while psc's are free."""
            gi = 0
            for ls in range(NL):
                for ocb in range(CB):
                    if gi < nalt:
                        ps = psc.tile([P, 2, LS], F32, tag="sc", name="acc_alt")[:, 0, :]
                    else:
                        ps = pacc.tile([P, LS], F32, tag="acc", name="acc")[:]
                    gi += 1
                    k = 0
                    for icb in range(CB):
                        for tap in range(3):
                            nc.tensor.matmul(
                                ps,
                                wt[:, icb, tap, ocb * P : (ocb + 1) * P],
                                src[icb][:, ls * LS + tap : ls * LS + tap + LS],
                                start=(k == 0),
                                stop=(k == 5),
                            )
                            k += 1
                    consume(ocb, ls, ps)

        # ------- per-sample state + fine-grained stage closures -------
        def make_sample(s):
            st = {}

            def load():
                # x rides the gpsimd SWDGE queue as 512KB descriptors (the
                # ~0.6us software descriptor gen is paid once per 1024 cols).
                # Do NOT route x through sync (queue stalls multi-us between
                # DMAs, ~55us end-to-end) or scalar (the HWDGE issue stalls
                # the ScalarE sequencer ~1.7us per descriptor, blocking the
                # groupnorm Ln/Exp behind it; measured +20us).  s0 is issued
                # first so its first groupnorm unblocks earliest.
                with nc.named_scope(f"s{s}_load"):
                    st["xp"] = alloc_padded("pad", padp)
                    for cb in range(CB):
                        for i in range(2):
                            nc.gpsimd.dma_start(
                                st["xp"][cb][:, 1 + i * 2 * LS : 1 + (i + 1) * 2 * LS],
                                x_d[s, cb * P : (cb + 1) * P, i * 2 * LS : (i + 1) * 2 * LS],
                            )
                return st

            def loadt():
                st["t2"] = t2p.tile([P, CB, 2], F32, tag="t2", name="t2")
                nc.gpsimd.dma_start(st["t2"][:], t_d[s])

            def gn1(rb, srckey, dstkey):
                def f():
                    a = alloc_padded("act", actp, BF16)
                    st[dstkey] = a
                    with nc.named_scope(f"s{s}_{rb}_gn1"):
                        st[f"corr_{rb}1"] = gn_relu(st[srckey], a, rb, 1)
                return f

            def conv1(rb, rbi, akey, hkey):
                def f():
                    h = alloc_padded("pad", padp)
                    st[hkey] = h
                    t2 = st["t2"]
                    corr = st[f"corr_{rb}1"]
                    with nc.named_scope(f"s{s}_{rb}_conv1"):
                        # fold the gn-bias conv correction into the t vector
                        tadj = persl.tile([P, 2], F32, tag="tadj", name="tadj")
                        nc.vector.tensor_tensor(out=tadj[:], in0=t2[:, :, rbi : rbi + 1], in1=corr[:], op=OP.add)

                        def eat1(ocb, ls, ps):
                            nc.vector.tensor_scalar_add(
                                h[ocb][:, 1 + ls * LS : 1 + (ls + 1) * LS], ps,
                                tadj[:, ocb : ocb + 1],
                            )
                        conv3(st[akey], w1_sb[rb], eat1, nalt=3 if rb == "r2" else 0)
                return f

            def gn2(rb, hkey, dstkey):
                def f():
                    a2 = alloc_padded("act", actp, BF16)
                    st[dstkey] = a2
                    with nc.named_scope(f"s{s}_{rb}_gn2"):
                        st[f"corr_{rb}2"] = gn_relu(st[hkey], a2, rb, 2)
                return f

            def conv2(rb, srckey, a2key, final):
                def f():
                    src = st[srckey]
                    corr = st[f"corr_{rb}2"]
                    res = None
                    if not final:
                        res = [seqp.tile([P, L], BF16, tag=f"res{cb}", name=f"res{cb}") for cb in range(CB)]
                        st["x1"] = res
                    with nc.named_scope(f"s{s}_{rb}_conv2"):
                        def eat2(ocb, ls, ps):
                            if rb in c2b_sb:
                                nc.vector.tensor_scalar_add(ps, ps, c2b_sb[rb][:, ocb : ocb + 1])
                            resid = _f(src[ocb][:, 1 + ls * LS : 1 + (ls + 1) * LS])
                            ccol = corr[:, ocb : ocb + 1]
                            if final:
                                ot = outp.tile([P, LS], F32, tag="out", name="ot")
                                nc.vector.scalar_tensor_tensor(
                                    out=ot[:], in0=ps, scalar=ccol, in1=resid,
                                    op0=OP.add, op1=OP.add,
                                )
                                nc.sync.dma_start(
                                    out_d[s, ocb * P : (ocb + 1) * P, ls * LS : (ls + 1) * LS], ot[:]
                                )
                            else:
                                nc.vector.scalar_tensor_tensor(
                                    out=res[ocb][:, ls * LS : (ls + 1) * LS],
                                    in0=ps, scalar=ccol, in1=resid,
                                    op0=OP.add, op1=OP.add,
                                )
                        conv3(st[a2key], w2_sb[rb], eat2)
                return f

            def kqv():
                x1 = st["x1"]
                kt = seqp.tile([P, CB, L], FP8, tag="kt", name="kt")
                qt = seqp.tile([P, CB, L], FP8, tag="qt", name="qt")
                vt = vtp.tile([P, NP, 2, C], FP8, tag="vt", name="vt")
                st["kt"], st["qt"], st["vt"] = kt, qt, vt
                with nc.named_scope(f"s{s}_kqv"):
                    cp = 0
                    for j, dst in ((0, kt), (1, qt)):
                        for ocb in range(CB):
                            off = j * C + ocb * P
                            for ls in range(NL):
                                if cp < 5:
                                    # dodge the attention-tail psav ring wait
                                    ps = psc.tile([P, 2, LS], F32, tag="sc", name="acc_alt")[:, 0, :]
                                else:
                                    ps = pacc.tile([P, LS], F32, tag="acc", name="acc")[:]
                                for icb in range(CB):
                                    nc.tensor.matmul(
                                        ps,
                                        wkqv_sb[:, icb, off : off + P],
                                        x1[icb][:, ls * LS : (ls + 1) * LS],
                                        start=(icb == 0),
                                        stop=(icb == 1),
                                    )
                                dsl = dst[:, ocb, ls * LS : (ls + 1) * LS]
                                cp += 1
                                # kqv drains ride ScalarE (idle during this
                                # phase; DVE is saturated by the conv eats +
                                # bn_stats running concurrently)
                                if linb_sb is not None:
                                    nc.scalar.activation(
                                        out=dsl, in_=ps, func=AF.Identity,
                                        bias=linb_sb[:, j * CB + ocb : j * CB + ocb + 1],
                                    )
                                else:
                                    nc.scalar.activation(out=dsl, in_=ps, func=AF.Copy)
                    # vT[l, c] (l on partitions) for the attention output matmul
                    for lb in range(KB):
                        ps = pacc.tile([P, LS], F32, tag="acc", name="acc")
                        for icb in range(CB):
                            nc.tensor.matmul(
                                ps[:, :C],
                                x1[icb][:, lb * P : (lb + 1) * P],
                                wkqv_sb[:, icb, 2 * C : 3 * C],
                                start=(icb == 0),
                                stop=(icb == 1),
                            )
                        # v bias (if any) is added to av after softmax: sum(a)=1
                        nc.scalar.activation(out=vt[:, lb // 2, lb % 2, :], in_=ps[:, :C], func=AF.Copy)

            def attn():
                kt, qt, vt = st["kt"], st["qt"], st["vt"]
                av = alloc_padded("pad", padp)
                st["av"] = av
                for qs in range(NL):
                    with nc.named_scope(f"s{s}_attn{qs}"):
                        dn = paux.tile([P, LS], F32, tag="aux", name="dn")
                        psav = [pacc.tile([P, LS], F32, tag="acc", name="psav") for _ in range(CB)]
                        for p in range(NP):
                            ex = expp.tile([P, 2, LS], FP8, tag="exp", name="exp")
                            sc = psc.tile([P, 2, LS], F32, tag="sc", name="sc")
                            for i in range(2):
                                kbg = 2 * p + i
                                nc.tensor.matmul(
                                    sc[:, i, :],
                                    kt[:, :, kbg * P : (kbg + 1) * P],
                                    qt[:, :, qs * LS : (qs + 1) * LS],
                                    start=True, stop=True, perf_mode=DR,
                                )
                            # one 1024-col exp per k-block pair
                            nc.scalar.activation(
                                out=ex[:, :, :], in_=sc[:, :, :], func=AF.Exp,
                                bias=shift_sb[:], scale=SCALE,
                            )
                            nc.tensor.matmul(
                                dn[0:1, :], ones8_sb[:, :, 0:1], ex[:],
                                start=(p == 0), stop=(p == NP - 1), perf_mode=DR,
                            )
                            for cb in range(CB):
                                nc.tensor.matmul(
                                    psav[cb][:],
                                    vt[:, p, :, cb * P : (cb + 1) * P],
                                    ex[:],
                                    start=(p == 0),
                                    stop=(p == NP - 1),
                                    perf_mode=DR,
                                )
                        # rd = 1/dn via Ln+Exp (reciprocal_approx_fast outputs
                        # f32 which the f32r broadcast matmul rejects)
                        lnd = rdsp.tile([1, LS], F32, tag="lnd", name="lnd")
                        nc.scalar.activation(out=lnd[:], in_=dn[0:1, :], func=AF.Ln)
                        rd = rdsp.tile([1, LS], F32R, tag="rd", name="rd")
                        nc.scalar.activation(out=rd[:], in_=lnd[:], func=AF.Exp, scale=-1.0)
                        # broadcast 1/denom across partitions via K=1 ones-matmul
                        # OVERWRITING the dn bank (dn is already consumed by the
                        # Ln): one paux alloc per qs keeps the aux ring free for
                        # the overlapped sample's groupnorm aggregation scratch
                        nc.tensor.matmul(dn[:], onesr_sb[:], rd[:], start=True, stop=True)
                        rdb = rdbp.tile([P, LS], F32, tag="rdbs", name="rdb")
                        nc.scalar.activation(out=rdb[:], in_=dn[:], func=AF.Copy)
                        for cb in range(CB):
                            avs = av[cb][:, 1 + qs * LS : 1 + (qs + 1) * LS]
                            nc.vector.tensor_tensor(out=avs, in0=psav[cb][:], in1=rdb[:], op=OP.mult)
                            if linb_sb is not None:
                                nc.vector.tensor_scalar_add(
                                    avs, _f(avs), linb_sb[:, 2 * CB + cb : 2 * CB + cb + 1]
                                )

            return {
                "st": st,
                "load": load,
                "loadt": loadt,
                "gn1": gn1("r1", "xp", "a"),
                "conv1": conv1("r1", 0, "a", "h"),
                "gn2": gn2("r1", "h", "a2"),
                "conv2": conv2("r1", "xp", "a2", final=False),
                "kqv": kqv,
                "attn": attn,
                "rgn1": gn1("r2", "av", "ra"),
                "rconv1": conv1("r2", 1, "ra", "rh"),
                "rgn2": gn2("r2", "rh", "ra2"),
                "rconv2": conv2("r2", "av", "ra2", final=True),
            }

        ph = [make_sample(s) for s in range(samples)]
        if samples == 2:
            s0, s1 = ph
            # interleave the two samples so every groupnorm stat chain and
            # softmax tail overlaps the other sample's matmuls
            s0["load"]()
            load_early_consts()
            s0["loadt"](); s1["loadt"]()
            s1["load"]()
            load_late_consts()
            warmup_pe(s0["st"]["xp"][0])
            s0["gn1"](); s1["gn1"]()
            s0["conv1"](); s0["gn2"]()
            s1["conv1"](); s1["gn2"]()
            s0["conv2"]()
            s1["conv2"]()
            s0["kqv"](); s0["attn"]()
            s1["kqv"]()
            s0["rgn1"]()
            s1["attn"]()
            s0["rconv1"]()
            s1["rgn1"]()
            s0["rgn2"]()
            s1["rconv1"]()
            s1["rgn2"]()
            s0["rconv2"]()
            s1["rconv2"]()
        else:
            for pi, p_ in enumerate(ph):
                p_["load"]()
                if pi == 0:
                    load_early_consts()
                p_["loadt"]()
                if pi == 0:
                    load_late_consts()
                    warmup_pe(p_["st"]["xp"][0])
                for k in ("gn1", "conv1", "gn2", "conv2", "kqv", "attn",
                          "rgn1", "rconv1", "rgn2", "rconv2"):
                    p_[k]()

    nc.finalize()
    return nc


def _pack_conv_w(w):
    """(O, I, 3) -> [P, icb, tap, oc] bf16."""
    w = np.asarray(w, dtype=np.float32)
    o, i, k = w.shape
    return np.ascontiguousarray(
        w.transpose(1, 2, 0).reshape(CB, P, 3, o).transpose(1, 0, 2, 3)
    ).astype(ml_dtypes.bfloat16)


def _pack_gn(v):
    """(256,) -> [P, CB]"""
    return np.ascontiguousarray(np.asarray(v, dtype=np.float32).reshape(CB, P).T)


def make_in_maps(inp, use_bias):
    """Host-side packing; returns the per-core input maps."""
    gind = np.zeros((P, GPB), np.float32)
    bind = np.zeros((CB, P, P), np.float32)
    for cc in range(P):
        gind[cc, cc // 8] = 0.125
        for cb in range(CB):
            bind[cb, cb * 32 + cc // 8, cc] = 1.0
    shared = {
        "wkqvt": np.ascontiguousarray(
            inp["lin_w"][:, :, 0].T.reshape(CB, P, 3 * C).transpose(1, 0, 2)
        ).astype(ml_dtypes.bfloat16),
        "gind": gind,
        "bind": bind,
        "ones8": np.ones((P, CB, 16), ml_dtypes.float8_e4m3),
        "onesr": np.ones((1, P), np.float32),
    }
    gnall = np.empty((P, CB, 8), np.float32)
    wsum = np.empty((P, CB, 4, C), np.float32)
    for rbi, rb in enumerate(("r1", "r2")):
        shared[f"{rb}_w1t"] = _pack_conv_w(inp[f"{rb}_c1_w"])
        shared[f"{rb}_w2t"] = _pack_conv_w(inp[f"{rb}_c2_w"])
        for ln in (1, 2):
            gnall[:, :, rbi * 4 + (ln - 1) * 2 + 0] = _pack_gn(inp[f"{rb}_gn{ln}_w"])
            gnall[:, :, rbi * 4 + (ln - 1) * 2 + 1] = _pack_gn(inp[f"{rb}_gn{ln}_b"])
            # wsum[icp, icb, j, oc] = sum_tap w[oc, ic, tap]
            w = np.asarray(inp[f"{rb}_c{ln}_w"], np.float32).sum(-1)  # (O, I)
            wsum[:, :, rbi * 2 + (ln - 1), :] = w.T.reshape(CB, P, C).transpose(1, 0, 2)
    shared["gnallt"] = gnall
    shared["wsumt"] = wsum
    if "c2b_r1" in use_bias:
        shared["r1_c2bs"] = _pack_gn(inp["r1_c2_b"])
    if "c2b_r2" in use_bias:
        shared["r2_c2bs"] = _pack_gn(inp["r2_c2_b"])
    if "linb" in use_bias:
        shared["lin_bs"] = np.ascontiguousarray(inp["lin_b"].reshape(3 * CB, P).T)

    # per-sample conv1 bias vector: t[s] + c1_b per res block -> [P, CB, 2]
    tfull = inp["t"][:, :, 0]  # (B, C)
    nb = inp["x"].shape[0]
    tv = np.empty((nb, P, CB, 2), np.float32)
    for rbi, rb in enumerate(("r1", "r2")):
        v = tfull + inp[f"{rb}_c1_b"][None, :]
        tv[:, :, :, rbi] = v.reshape(nb, CB, P).transpose(0, 2, 1)

    in_maps = []
    for c in range(NCORES):
        sl = slice(S * c, S * (c + 1))
        m = dict(shared)
        m["x"] = inp["x"][sl]
        m["tv"] = np.ascontiguousarray(tv[sl])
        in_maps.append(m)
    return in_maps


_CACHE = {}


def kernel(**inputs):
    inp = {k: np.ascontiguousarray(np.asarray(v, dtype=np.float32)) for k, v in inputs.items()}

    use_bias = []
    if np.any(inp["r1_c2_b"]):
        use_bias.append("c2b_r1")
    if np.any(inp["r2_c2_b"]):
        use_bias.append("c2b_r2")
    if np.any(inp["lin_b"]):
        use_bias.append("linb")
    use_bias = tuple(use_bias)

    if ("nc", use_bias) not in _CACHE:
        _CACHE[("nc", use_bias)] = build_program(S, use_bias)
    nc = _CACHE[("nc", use_bias)]

    in_maps = make_in_maps(inp, use_bias)
    res = _bu.run_bass_kernel_spmd(nc, in_maps, core_ids=list(range(NCORES)))
    out = np.concatenate([res.results[c]["out"] for c in range(NCORES)], axis=0)
    return out.astype(np.float32)



# revision 38
# speedup vs baseline: 1.0763x; 1.0498x over previous
"""Trainium2 Bass kernel for nn_MidAttnBlock (res-block -> full LxL attention -> res-block).

Contract: kernel(**inputs) takes the FULL inputs of reference.setup_inputs()
(x: (16,256,2048) f32, t: (16,256,1) f32, plus conv/groupnorm/linear params)
and returns the FULL (16,256,2048) f32 output.  Data-parallel over batch on
8 NeuronCores, 2 samples per core; each core runs an identical Bass program.

Convs and the kqv projection run in bf16 (full-rate PE, half the SBUF/HBM
of f32r).  The attention (scores, softmax weights, A@V) runs in fp8e4 with
DoubleRow matmuls (256-deep contraction per PE pass).  ~7.5e-3 end-to-end
relative error (threshold 2e-2).  exp is computed as exp(s/16 - 4) so the
softmax weights fit fp8e4's range; the shift cancels in the normalization.

The groupnorm relu applies run on the Vector engine as one
tensor_scalar((x*s) max -b2) per chunk: the activation tiles store
relu(gn(x)) - b2 with pad columns = -b2, and the following conv adds back
the per-output-channel constant corr = Wsum @ b2 (folded into the t-vector
add for conv1 and the residual add for conv2).  This keeps ScalarE free for
the attention exps and the kqv PSUM drains.

The two samples on each core are issued interleaved
(s0:r1,kqv | s1:r1 | s0:attn | s1:kqv | s0:r2 | s1:attn | s1:r2) so the
groupnorm stat chains and softmax tails of one sample overlap the other
sample's matmuls.

Self-contained: all shapes/sharding hardcoded.
"""

import json as _json

import ml_dtypes
import numpy as np

import concourse.bass as bass
import concourse.bass2jax as _b2j
import concourse.bass_utils as _bu
import concourse.tile as tile
from concourse import mybir
from concourse.vector_clock import ScopedClock, VectorClock


def _split_bir_waits(bir_json):
    """The walrus_driver in this container encodes at most ONE sync-wait per
    instruction (and none on Drain).  Tile's sem assigner attaches several.
    Rewrite the BIR: excess waits move to single-wait NoOps inserted directly
    before the instruction on the same engine."""
    m = _json.loads(bir_json)
    ctr = 0
    for fn in m.get("functions", []):
        for bb in fn.get("blocks", []):
            out = []
            for ins in bb.get("instructions", []):
                si = ins.get("sync_info")
                waits = (si or {}).get("on_wait") or []
                keep = 0 if ins.get("opcode") == "Drain" else 1
                if len(waits) > keep:
                    nmove = len(waits) - keep
                    for w in waits[:nmove]:
                        ctr += 1
                        out.append({
                            "debug": ins.get("debug", 0),
                            "engine": ins["engine"],
                            "ins": [],
                            "name": f"{ins['name']}-wsp{ctr}",
                            "opcode": "NoOp",
                            "outs": [],
                            "sync_info": {"on_update": [], "on_wait": [w]},
                        })
                    si["on_wait"] = waits[nmove:]
                out.append(ins)
            bb["instructions"] = out
    return _json.dumps(m).encode()


_orig_compile_bir_kernel = _bu.compile_bir_kernel


def _compile_bir_splitwaits(bir_json, tmpdir, neff_name="file.neff"):
    return _orig_compile_bir_kernel(_split_bir_waits(bir_json), tmpdir, neff_name)


if getattr(_bu.compile_bir_kernel, "__name__", "") != "_compile_bir_splitwaits":
    _bu.compile_bir_kernel = _compile_bir_splitwaits
    _b2j.compile_bir_kernel = _compile_bir_splitwaits


F32 = mybir.dt.float32
F32R = mybir.dt.float32r
BF16 = mybir.dt.bfloat16
FP8 = mybir.dt.float8e4
AF = mybir.ActivationFunctionType
OP = mybir.AluOpType
DR = mybir.MatmulPerfMode.DoubleRow

P = 128          # partitions
C = 256          # channels
CB = 2           # channel blocks of 128
L = 2048         # sequence length
LS = 512         # l-slice (matmul moving dim)
NL = L // LS     # 4 slices
KB = L // P      # 16 k-blocks for attention
NP = KB // 2     # 8 k-block pairs (DoubleRow)
GPB = 16         # groups per channel-block (32 groups, 8 ch each)
EPS = 1e-5
S = 2            # samples per core
NCORES = 8
SCALE = 1.0 / 16.0   # 1/sqrt(C)
ESHIFT = -4.0        # exp(s*SCALE + ESHIFT): keeps softmax weights in fp8e4 range


class _TileContextPatched(tile.TileContext):
    """TileContext whose kernel-tail drain carries no sem waits (the container
    walrus rejects waits on Drain); one SP NOP per proc carries them instead."""

    def _drain_and_barrier(self, tick_clock, wait_clock):
        gc = tick_clock.global_clock
        n = len(gc)
        for p in range(n):
            v = gc[p]
            if v > 0:
                vec = [0] * n
                vec[p] = v
                nop = self.nc.sync.nop()
                wait_clock.add_sem_waits(nop.ins, ScopedClock({None: VectorClock(vec)}))
        self.nc.sync.drain()
        self.nc.all_engine_barrier()
        assert self.sems is not None
        popped = self.nc._tile_sem_poison_stack.pop()
        assert popped is self._sem_poison
        self.nc.clear_and_free_semaphores(list(self.sems.allocated().values()))
        self.nc.all_engine_barrier()


def _f(ap):
    """Read an f32r tile as plain f32 (same bits) for VectorE/ScalarE inputs."""
    return ap.bitcast(F32)


def build_program(samples=S, use_bias=()):
    """Build the per-core Bass program (identical on all cores).

    use_bias: subset of {"c2b_r1", "c2b_r2", "linb"} enabling extra adds for
    biases that setup_inputs() keeps at zero.
    """
    nc = bass.Bass()

    # ---- DRAM I/O (per core) ----
    x_d = nc.dram_tensor("x", (samples, C, L), F32R, kind="ExternalInput")
    # t + conv1 bias, host-packed [samples, P, CB, 2(resblock)]
    t_d = nc.dram_tensor("tv", (samples, P, CB, 2), F32, kind="ExternalInput")
    w_conv = {}
    for rb in ("r1", "r2"):
        # host-packed [P(ic within block), icb, tap, oc]
        w_conv[rb, 1] = nc.dram_tensor(f"{rb}_w1t", (P, CB, 3, C), BF16, kind="ExternalInput")
        w_conv[rb, 2] = nc.dram_tensor(f"{rb}_w2t", (P, CB, 3, C), BF16, kind="ExternalInput")
    wkqv_d = nc.dram_tensor("wkqvt", (P, CB, 3 * C), BF16, kind="ExternalInput")
    # tap-summed conv weights for the groupnorm-bias correction:
    # wsum[icp, icb, j, oc] = sum_tap w_j[oc, ic, tap], j in (r1c1, r1c2, r2c1, r2c2)
    wsum_d = nc.dram_tensor("wsumt", (P, CB, 4, C), F32R, kind="ExternalInput")
    # all 8 groupnorm affine vectors in one tensor: [P, CB, rb*4+(ln-1)*2+wb]
    gnall_d = nc.dram_tensor("gnallt", (P, CB, 8), F32, kind="ExternalInput")
    c2b_d = {}
    if "c2b_r1" in use_bias:
        c2b_d["r1"] = nc.dram_tensor("r1_c2bs", (P, CB), F32, kind="ExternalInput")
    if "c2b_r2" in use_bias:
        c2b_d["r2"] = nc.dram_tensor("r2_c2bs", (P, CB), F32, kind="ExternalInput")
    linb_d = None
    if "linb" in use_bias:
        linb_d = nc.dram_tensor("lin_bs", (P, 3 * CB), F32, kind="ExternalInput")
    gind_d = nc.dram_tensor("gind", (P, GPB), F32R, kind="ExternalInput")  # 1/8 group indicator
    bind_d = nc.dram_tensor("bind", (CB, P, P), F32R, kind="ExternalInput")    # group->channel broadcast
    ones8_d = nc.dram_tensor("ones8", (P, CB, 16), FP8, kind="ExternalInput")
    onesr_d = nc.dram_tensor("onesr", (1, P), F32R, kind="ExternalInput")
    out_d = nc.dram_tensor("out", (samples, C, L), F32, kind="ExternalOutput")

    with _TileContextPatched(nc) as tc, \
         tc.tile_pool(name="consts", bufs=1) as consts, \
         tc.tile_pool(name="padp", bufs=4) as padp, \
         tc.tile_pool(name="actp", bufs=2) as actp, \
         tc.tile_pool(name="seqp", bufs=2) as seqp, \
         tc.tile_pool(name="vtp", bufs=1) as vtp, \
         tc.tile_pool(name="expp", bufs=3) as expp, \
         tc.tile_pool(name="outp", bufs=2) as outp, \
         tc.tile_pool(name="rdbp", bufs=2) as rdbp, \
         tc.tile_pool(name="rdsp", bufs=2) as rdsp, \
         tc.tile_pool(name="small", bufs=4) as small, \
         tc.tile_pool(name="persl", bufs=12) as persl, \
         tc.tile_pool(name="t2p", bufs=2) as t2p, \
         tc.tile_pool(name="pacc", bufs=3, space="PSUM") as pacc, \
         tc.tile_pool(name="psc", bufs=2, space="PSUM") as psc, \
         tc.tile_pool(name="paux", bufs=1, space="PSUM") as paux:

        # ---- persistent constants / weights in SBUF ----
        # All const loads ride the gpsimd SWDGE queue (descriptor gen is
        # ~0.6us each, so the count is kept low and ordered so the tensors
        # gating the pipeline head land first); x/t ride the fast ScalarE
        # HWDGE queue concurrently.
        gind_sb = consts.tile([P, GPB], F32R, tag="gind", name="gind")
        gnall_sb = consts.tile([P, CB, 8], F32, tag="gnall", name="gnall")
        wsum_sb = consts.tile([P, CB, 4, C], F32R, tag="wsum", name="wsum")
        bind_sb = consts.tile([P, CB, P], F32R, tag="bind", name="bind")

        def load_early_consts():
            # issued between the s0 and s1 x loads: everything the s0
            # groupnorm chain + first conv needs, in dependency order
            nc.gpsimd.dma_start(gind_sb[:], gind_d[:])
            nc.gpsimd.dma_start(gnall_sb[:], gnall_d[:])
            nc.gpsimd.dma_start(wsum_sb[:, :, 0:2, :], wsum_d[:, :, 0:2, :])
            nc.gpsimd.dma_start(bind_sb[:], bind_d.rearrange("cb p c -> p cb c"))
            nc.gpsimd.dma_start(w1_sb["r1"][:], w_conv["r1", 1][:])

        def gnp(rb, ln, wb, cb):
            idx = (0 if rb == "r1" else 4) + (ln - 1) * 2 + (0 if wb == "w" else 1)
            return gnall_sb[:, cb, idx : idx + 1]

        w1_sb = {}
        w2_sb = {}
        late_consts = []
        for rb in ("r1", "r2"):
            w1_sb[rb] = consts.tile([P, CB, 3, C], BF16, tag=f"w1_{rb}", name=f"w1_{rb}")
            w2_sb[rb] = consts.tile([P, CB, 3, C], BF16, tag=f"w2_{rb}", name=f"w2_{rb}")
            if rb == "r1":
                late_consts.insert(0, (w2_sb[rb][:], w_conv[rb, 2][:]))
            else:
                late_consts.append((w1_sb[rb][:], w_conv[rb, 1][:]))
                late_consts.append((w2_sb[rb][:], w_conv[rb, 2][:]))
        wkqv_sb = consts.tile([P, CB, 3 * C], BF16, tag="wkqv", name="wkqv")
        late_consts.insert(1, (wkqv_sb[:], wkqv_d[:]))
        late_consts.append((wsum_sb[:, :, 2:4, :], wsum_d[:, :, 2:4, :]))
        c2b_sb = {}
        for rb, d in c2b_d.items():
            c2b_sb[rb] = consts.tile([P, CB], F32, tag=f"c2b_{rb}", name=f"c2b_{rb}")
            late_consts.append((c2b_sb[rb][:], d[:]))
        linb_sb = None
        if linb_d is not None:
            linb_sb = consts.tile([P, 3 * CB], F32, tag="linb", name="linb")
            late_consts.append((linb_sb[:], linb_d[:]))
        ones8_sb = consts.tile([P, CB, 16], FP8, tag="ones8", name="ones8")
        late_consts.append((ones8_sb[:], ones8_d[:]))
        onesr_sb = consts.tile([1, P], F32R, tag="onesr", name="onesr")
        late_consts.append((onesr_sb[:], onesr_d[:]))

        def load_late_consts():
            # issued after the x loads so they don't contend for HBM bandwidth
            # ahead of the first groupnorm
            for ap, src in late_consts:
                nc.gpsimd.dma_start(ap, src)
        eps_sb = consts.tile([P, 1], F32, tag="eps", name="eps")
        nc.vector.memset(eps_sb[:], EPS)
        shift_sb = consts.tile([P, 1], F32, tag="shift", name="shift")
        nc.vector.memset(shift_sb[:], ESHIFT)
        zero2 = consts.tile([P, 2], F32, tag="zero2", name="zero2")
        nc.vector.memset(zero2[:], 0.0)

        def warmup_pe(xt):
            # ~3us of tiny back-to-back matmuls (on the first-loaded x chunk,
            # values irrelevant) while the rest of the DMAs land, so the PE
            # p-state is at full clock when the first conv group issues
            wk = paux.tile([P, LS], F32, tag="aux", name="warm")
            for _ in range(96):
                nc.tensor.matmul(wk[:GPB, 0:GPB], xt[:, 1:17], xt[:, 1:17], start=True, stop=True)

        def alloc_padded(tag, pool, dt=F32R):
            """[P, L+2] tile per channel block; data cols [1, L+1).

            Edge columns are only meaningful for conv sources (the gn_relu
            destinations), where gn_relu writes them to -b2."""
            return [
                pool.tile([P, L + 2], dt, tag=f"{tag}{cb}", name=f"{tag}{cb}")
                for cb in range(CB)
            ]

        def gn_relu(src, dst, rb, ln):
            """dst = relu(groupnorm(src)*w + b) - b2, with pad cols = -b2.

            b2 is the effective per-channel bias (b - m*rstd*w); subtracting it
            turns the ScalarE relu into one DVE tensor_scalar (x*s max -b2),
            and padding with -b2 keeps the following conv exact up to a
            per-output-channel constant corr = Wsum @ b2 that the conv
            consumer adds back (returned here as a [P, CB] column tile)."""
            gp = paux.tile([P, LS], F32, tag="aux", name="gbc")
            for cb in range(CB):
                stats = small.tile([P, NL, 6], F32, tag="stats", name="stats")
                for i in range(NL):
                    nc.vector.bn_stats(out=stats[:, i, :], in_=_f(src[cb][:, 1 + i * LS : 1 + (i + 1) * LS]))
                mv = small.tile([P, 2], F32, tag="mv", name="mv")
                nc.vector.bn_aggr(out=mv[:], in_=stats[:])
                # tmp = [mean_c, E[x^2]_c]  (f32r: feeds the aggregation matmul)
                tmp = small.tile([P, 2], F32R, tag="tmp", name="tmp")
                nc.vector.tensor_copy(out=tmp[:, 0:1], in_=mv[:, 0:1])
                nc.vector.tensor_tensor(out=tmp[:, 1:2], in0=mv[:, 0:1], in1=mv[:, 0:1], op=OP.mult)
                nc.vector.tensor_tensor(out=tmp[:, 1:2], in0=_f(tmp[:, 1:2]), in1=mv[:, 1:2], op=OP.add)
                nc.tensor.matmul(gp[:GPB, 2 * cb : 2 * cb + 2], gind_sb[:], tmp[:], start=True, stop=True)
            # merged group stats; block-cb groups live at partition offset 32*cb
            NG = 32 * CB
            gs = small.tile([NG, 2], F32, tag="gs", name="gs")
            nc.vector.tensor_copy(out=gs[:], in_=zero2[:NG])
            for cb in range(CB):
                nc.vector.tensor_copy(out=gs[cb * 32 : cb * 32 + GPB, :], in_=gp[:GPB, 2 * cb : 2 * cb + 2])
            var = small.tile([NG, 1], F32, tag="var", name="var")
            nc.vector.tensor_tensor(out=var[:], in0=gs[:, 0:1], in1=gs[:, 0:1], op=OP.mult)
            nc.vector.tensor_tensor(out=var[:], in0=gs[:, 1:2], in1=var[:], op=OP.subtract)
            nc.scalar.activation(out=var[:], in_=var[:], func=AF.Ln, bias=eps_sb[:NG])
            rstd = small.tile([NG, 1], F32, tag="rstd", name="rstd")
            nc.scalar.activation(out=rstd[:], in_=var[:], func=AF.Exp, scale=-0.5)
            # pack [rstd_g, +m_g], zero-extended to 128 partitions
            gpk = small.tile([P, 2], F32R, tag="gpk", name="gpk")
            nc.vector.tensor_copy(out=gpk[:], in_=zero2[:])
            nc.vector.tensor_copy(out=gpk[:NG, 0:1], in_=rstd[:])
            nc.vector.tensor_copy(out=gpk[:NG, 1:2], in_=gs[:, 0:1])
            sbs = []
            for cb in range(CB):
                # broadcast to channels: bc[c, :] = [rstd_g(c), m_g(c)]
                nc.tensor.matmul(gp[:, 4 + 2 * cb : 6 + 2 * cb], bind_sb[:, cb, :], gpk[:], start=True, stop=True)
                # sb = [s, -b2] = [rstd*w, m*s - b]  (f32r: feeds the corr matmul)
                sb = small.tile([P, 2], F32R, tag="sb", name="sb")
                nc.vector.tensor_scalar_mul(sb[:, 0:1], gp[:, 4 + 2 * cb : 5 + 2 * cb], gnp(rb, ln, "w", cb))
                nc.vector.tensor_tensor(out=sb[:, 1:2], in0=gp[:, 5 + 2 * cb : 6 + 2 * cb], in1=_f(sb[:, 0:1]), op=OP.mult)
                nc.vector.tensor_scalar_sub(sb[:, 1:2], _f(sb[:, 1:2]), gnp(rb, ln, "b", cb))
                sbs.append(sb)
                # conv pad columns hold -b2 (the stored value of a zero activation)
                nc.vector.tensor_copy(out=dst[cb][:, 0:1], in_=sb[:, 1:2])
                nc.vector.tensor_copy(out=dst[cb][:, L + 1 : L + 2], in_=sb[:, 1:2])
            # corr[oc] = sum_ic Wsum[ic, oc] * b2[ic], directly as per-ocb
            # columns: lhsT = Wsum block, rhs = -b2 column, negated in the
            # PSUM->SBUF copy
            j = (0 if rb == "r1" else 2) + (ln - 1)
            for ocb in range(CB):
                for icb in range(CB):
                    # moving operand is the whole [s, -b2] sb tile (f32r needs
                    # even offset/width); column 0 of the product is unused
                    nc.tensor.matmul(
                        gp[:, 8 + 2 * ocb : 10 + 2 * ocb],
                        wsum_sb[:, icb, j, ocb * P : (ocb + 1) * P],
                        sbs[icb][:],
                        start=(icb == 0), stop=(icb == 1),
                    )
            corr = persl.tile([P, 2], F32, tag="corr", name="corr")
            for ocb in range(CB):
                nc.vector.tensor_scalar_mul(corr[:, ocb : ocb + 1], gp[:, 9 + 2 * ocb : 10 + 2 * ocb], -1.0)
            # apply on DVE in NL chunks, cb-interleaved so the first conv
            # group (which reads both cb blocks) unblocks earliest
            for i in range(NL):
                for cb in range(CB):
                    nc.vector.tensor_scalar(
                        dst[cb][:, 1 + i * LS : 1 + (i + 1) * LS],
                        _f(src[cb][:, 1 + i * LS : 1 + (i + 1) * LS]),
                        _f(sbs[cb][:, 0:1]),
                        _f(sbs[cb][:, 1:2]),
                        OP.mult,
                        OP.max,
                    )
            return corr

        def conv3(src, wt, consume, nalt=0):
            """3-tap conv over padded f32r src; consume(ocb, ls, psum_ap).

            The first `nalt` output groups draw their PSUM from the psc ring
            instead of pacc — after an attention phase, pacc's slots are still
            pinned by the softmax tail (psav reads), while psc's are free."""
            gi = 0
            for ls in range(NL):
                for ocb in range(CB):
                    if gi < nalt:
                        ps = psc.tile([P, 2, LS], F32, tag="sc", name="acc_alt")[:, 0, :]
                    else:
                        ps = pacc.tile([P, LS], F32, tag="acc", name="acc")[:]
                    gi += 1
                    k = 0
                    for icb in range(CB):
                        for tap in range(3):
                            nc.tensor.matmul(
                                ps,
                                wt[:, icb, tap, ocb * P : (ocb + 1) * P],
                                src[icb][:, ls * LS + tap : ls * LS + tap + LS],
                                start=(k == 0),
                                stop=(k == 5),
                            )
                            k += 1
                    consume(ocb, ls, ps)

        # ------- per-sample state + fine-grained stage closures -------
        def make_sample(s):
            st = {}

            def load():
                # x rides the gpsimd SWDGE queue as 512KB descriptors (the
                # ~0.6us software descriptor gen is paid once per 1024 cols).
                # Do NOT route x through sync (queue stalls multi-us between
                # DMAs, ~55us end-to-end) or scalar (the HWDGE issue stalls
                # the ScalarE sequencer ~1.7us per descriptor, blocking the
                # groupnorm Ln/Exp behind it; measured +20us).  s0 is issued
                # first so its first groupnorm unblocks earliest.
                with nc.named_scope(f"s{s}_load"):
                    st["xp"] = alloc_padded("pad", padp)
                    for cb in range(CB):
                        for i in range(2):
                            nc.gpsimd.dma_start(
                                st["xp"][cb][:, 1 + i * 2 * LS : 1 + (i + 1) * 2 * LS],
                                x_d[s, cb * P : (cb + 1) * P, i * 2 * LS : (i + 1) * 2 * LS],
                            )
                return st

            def loadt():
                st["t2"] = t2p.tile([P, CB, 2], F32, tag="t2", name="t2")
                nc.gpsimd.dma_start(st["t2"][:], t_d[s])

            def gn1(rb, srckey, dstkey):
                def f():
                    a = alloc_padded("act", actp, BF16)
                    st[dstkey] = a
                    with nc.named_scope(f"s{s}_{rb}_gn1"):
                        st[f"corr_{rb}1"] = gn_relu(st[srckey], a, rb, 1)
                return f

            def conv1(rb, rbi, akey, hkey):
                def f():
                    h = alloc_padded("pad", padp)
                    st[hkey] = h
                    t2 = st["t2"]
                    corr = st[f"corr_{rb}1"]
                    with nc.named_scope(f"s{s}_{rb}_conv1"):
                        # fold the gn-bias conv correction into the t vector
                        tadj = persl.tile([P, 2], F32, tag="tadj", name="tadj")
                        nc.vector.tensor_tensor(out=tadj[:], in0=t2[:, :, rbi : rbi + 1], in1=corr[:], op=OP.add)

                        def eat1(ocb, ls, ps):
                            # ScalarE drain: keeps DVE free so the following
                            # groupnorm's bn_stats/chain (which read h) finish
                            # right after the conv instead of queueing behind
                            # 8 DVE drains
                            nc.scalar.activation(
                                out=h[ocb][:, 1 + ls * LS : 1 + (ls + 1) * LS],
                                in_=ps, func=AF.Identity,
                                bias=tadj[:, ocb : ocb + 1],
                            )
                        conv3(st[akey], w1_sb[rb], eat1, nalt=3 if rb == "r2" else 0)
                return f

            def gn2(rb, hkey, dstkey):
                def f():
                    a2 = alloc_padded("act", actp, BF16)
                    st[dstkey] = a2
                    with nc.named_scope(f"s{s}_{rb}_gn2"):
                        st[f"corr_{rb}2"] = gn_relu(st[hkey], a2, rb, 2)
                return f

            def conv2(rb, srckey, a2key, final):
                def f():
                    src = st[srckey]
                    corr = st[f"corr_{rb}2"]
                    res = None
                    if not final:
                        res = [seqp.tile([P, L], BF16, tag=f"res{cb}", name=f"res{cb}") for cb in range(CB)]
                        st["x1"] = res
                    with nc.named_scope(f"s{s}_{rb}_conv2"):
                        def eat2(ocb, ls, ps):
                            if rb in c2b_sb:
                                nc.vector.tensor_scalar_add(ps, ps, c2b_sb[rb][:, ocb : ocb + 1])
                            resid = _f(src[ocb][:, 1 + ls * LS : 1 + (ls + 1) * LS])
                            ccol = corr[:, ocb : ocb + 1]
                            if final:
                                ot = outp.tile([P, LS], F32, tag="out", name="ot")
                                nc.vector.scalar_tensor_tensor(
                                    out=ot[:], in0=ps, scalar=ccol, in1=resid,
                                    op0=OP.add, op1=OP.add,
                                )
                                nc.sync.dma_start(
                                    out_d[s, ocb * P : (ocb + 1) * P, ls * LS : (ls + 1) * LS], ot[:]
                                )
                            else:
                                nc.vector.scalar_tensor_tensor(
                                    out=res[ocb][:, ls * LS : (ls + 1) * LS],
                                    in0=ps, scalar=ccol, in1=resid,
                                    op0=OP.add, op1=OP.add,
                                )
                        conv3(st[a2key], w2_sb[rb], eat2)
                return f

            def kqv():
                x1 = st["x1"]
                kt = seqp.tile([P, CB, L], FP8, tag="kt", name="kt")
                qt = seqp.tile([P, CB, L], FP8, tag="qt", name="qt")
                vt = vtp.tile([P, NP, 2, C], FP8, tag="vt", name="vt")
                st["kt"], st["qt"], st["vt"] = kt, qt, vt
                with nc.named_scope(f"s{s}_kqv"):
                    cp = 0
                    for j, dst in ((0, kt), (1, qt)):
                        for ocb in range(CB):
                            off = j * C + ocb * P
                            for ls in range(NL):
                                if cp < 5:
                                    # dodge the attention-tail psav ring wait
                                    ps = psc.tile([P, 2, LS], F32, tag="sc", name="acc_alt")[:, 0, :]
                                else:
                                    ps = pacc.tile([P, LS], F32, tag="acc", name="acc")[:]
                                for icb in range(CB):
                                    nc.tensor.matmul(
                                        ps,
                                        wkqv_sb[:, icb, off : off + P],
                                        x1[icb][:, ls * LS : (ls + 1) * LS],
                                        start=(icb == 0),
                                        stop=(icb == 1),
                                    )
                                dsl = dst[:, ocb, ls * LS : (ls + 1) * LS]
                                cp += 1
                                # kqv drains ride ScalarE (idle during this
                                # phase; DVE is saturated by the conv eats +
                                # bn_stats running concurrently)
                                if linb_sb is not None:
                                    nc.scalar.activation(
                                        out=dsl, in_=ps, func=AF.Identity,
                                        bias=linb_sb[:, j * CB + ocb : j * CB + ocb + 1],
                                    )
                                else:
                                    nc.scalar.activation(out=dsl, in_=ps, func=AF.Copy)
                    # vT[l, c] (l on partitions) for the attention output matmul
                    for lb in range(KB):
                        ps = pacc.tile([P, LS], F32, tag="acc", name="acc")
                        for icb in range(CB):
                            nc.tensor.matmul(
                                ps[:, :C],
                                x1[icb][:, lb * P : (lb + 1) * P],
                                wkqv_sb[:, icb, 2 * C : 3 * C],
                                start=(icb == 0),
                                stop=(icb == 1),
                            )
                        # v bias (if any) is added to av after softmax: sum(a)=1
                        nc.scalar.activation(out=vt[:, lb // 2, lb % 2, :], in_=ps[:, :C], func=AF.Copy)

            def attn():
                kt, qt, vt = st["kt"], st["qt"], st["vt"]
                av = alloc_padded("pad", padp)
                st["av"] = av
                # The softmax tail of qs is emitted INSIDE qs+1's pair loop so
                # the in-order PE never waits on it: Ln/rd (ScalarE) go right
                # after qs+1's first exp; the broadcast matmul + rdb + mults go
                # just before pair 2.  The broadcast draws a scores-ring slot
                # (free by then); the dn bank frees at the Ln read, so qs+1's
                # dn alloc never waits the tail.
                tail1 = [None]
                tail2 = [None]
                for qs in range(NL):
                    with nc.named_scope(f"s{s}_attn{qs}"):
                        dn = paux.tile([P, LS], F32, tag="aux", name="dn")
                        psav = [pacc.tile([P, LS], F32, tag="acc", name="psav") for _ in range(CB)]
                        for p in range(NP):
                            ex = expp.tile([P, 2, LS], FP8, tag="exp", name="exp")
                            if p == 2 and tail2[0] is not None:
                                tail2[0](); tail2[0] = None
                            sc = psc.tile([P, 2, LS], F32, tag="sc", name="sc")
                            for i in range(2):
                                kbg = 2 * p + i
                                nc.tensor.matmul(
                                    sc[:, i, :],
                                    kt[:, :, kbg * P : (kbg + 1) * P],
                                    qt[:, :, qs * LS : (qs + 1) * LS],
                                    start=True, stop=True, perf_mode=DR,
                                )
                            # one 1024-col exp per k-block pair
                            nc.scalar.activation(
                                out=ex[:, :, :], in_=sc[:, :, :], func=AF.Exp,
                                bias=shift_sb[:], scale=SCALE,
                            )
                            if p == 0 and tail1[0] is not None:
                                tail1[0](); tail1[0] = None
                            nc.tensor.matmul(
                                dn[0:1, :], ones8_sb[:, :, 0:1], ex[:],
                                start=(p == 0), stop=(p == NP - 1), perf_mode=DR,
                            )
                            for cb in range(CB):
                                nc.tensor.matmul(
                                    psav[cb][:],
                                    vt[:, p, :, cb * P : (cb + 1) * P],
                                    ex[:],
                                    start=(p == 0),
                                    stop=(p == NP - 1),
                                    perf_mode=DR,
                                )

                    def mk_tails(qs=qs, dn=dn, psav=psav):
                        rd = rdsp.tile([1, LS], F32R, tag="rd", name="rd")

                        def t1():
                            lnd = rdsp.tile([1, LS], F32, tag="lnd", name="lnd")
                            nc.scalar.activation(out=lnd[:], in_=dn[0:1, :], func=AF.Ln)
                            nc.scalar.activation(out=rd[:], in_=lnd[:], func=AF.Exp, scale=-1.0)

                        def t2():
                            with nc.named_scope(f"s{s}_atail{qs}"):
                                rbt = psc.tile([P, 2, LS], F32, tag="sc", name="rb_ps")[:, 0, :]
                                nc.tensor.matmul(rbt, onesr_sb[:], rd[:], start=True, stop=True)
                                rdb = rdbp.tile([P, LS], F32, tag="rdbs", name="rdb")
                                nc.scalar.activation(out=rdb[:], in_=rbt, func=AF.Copy)
                                for cb in range(CB):
                                    avs = av[cb][:, 1 + qs * LS : 1 + (qs + 1) * LS]
                                    nc.vector.tensor_tensor(out=avs, in0=psav[cb][:], in1=rdb[:], op=OP.mult)
                                    if linb_sb is not None:
                                        nc.vector.tensor_scalar_add(
                                            avs, _f(avs), linb_sb[:, 2 * CB + cb : 2 * CB + cb + 1]
                                        )
                        return t1, t2

                    tail1[0], tail2[0] = mk_tails()
                # last qs: emit the remaining tail immediately
                tail1[0]()
                tail2[0]()

            return {
                "st": st,
                "load": load,
                "loadt": loadt,
                "gn1": gn1("r1", "xp", "a"),
                "conv1": conv1("r1", 0, "a", "h"),
                "gn2": gn2("r1", "h", "a2"),
                "conv2": conv2("r1", "xp", "a2", final=False),
                "kqv": kqv,
                "attn": attn,
                "rgn1": gn1("r2", "av", "ra"),
                "rconv1": conv1("r2", 1, "ra", "rh"),
                "rgn2": gn2("r2", "rh", "ra2"),
                "rconv2": conv2("r2", "av", "ra2", final=True),
            }

        ph = [make_sample(s) for s in range(samples)]
        if samples == 2:
            s0, s1 = ph
            # interleave the two samples so every groupnorm stat chain and
            # softmax tail overlaps the other sample's matmuls
            s0["load"]()
            load_early_consts()
            s0["loadt"](); s1["loadt"]()
            s1["load"]()
            load_late_consts()
            warmup_pe(s0["st"]["xp"][0])
            s0["gn1"](); s1["gn1"]()
            s0["conv1"](); s0["gn2"]()
            s1["conv1"](); s1["gn2"]()
            s0["conv2"]()
            s1["conv2"]()
            s0["kqv"](); s0["attn"]()
            s1["kqv"]()
            s0["rgn1"]()
            s1["attn"]()
            s0["rconv1"]()
            s1["rgn1"]()
            s0["rgn2"]()
            s1["rconv1"]()
            s1["rgn2"]()
            s0["rconv2"]()
            s1["rconv2"]()
        else:
            for pi, p_ in enumerate(ph):
                p_["load"]()
                if pi == 0:
                    load_early_consts()
                p_["loadt"]()
                if pi == 0:
                    load_late_consts()
                    warmup_pe(p_["st"]["xp"][0])
                for k in ("gn1", "conv1", "gn2", "conv2", "kqv", "attn",
                          "rgn1", "rconv1", "rgn2", "rconv2"):
                    p_[k]()

    nc.finalize()
    return nc


def _pack_conv_w(w):
    """(O, I, 3) -> [P, icb, tap, oc] bf16."""
    w = np.asarray(w, dtype=np.float32)
    o, i, k = w.shape
    return np.ascontiguousarray(
        w.transpose(1, 2, 0).reshape(CB, P, 3, o).transpose(1, 0, 2, 3)
    ).astype(ml_dtypes.bfloat16)


def _pack_gn(v):
    """(256,) -> [P, CB]"""
    return np.ascontiguousarray(np.asarray(v, dtype=np.float32).reshape(CB, P).T)


def make_in_maps(inp, use_bias):
    """Host-side packing; returns the per-core input maps."""
    gind = np.zeros((P, GPB), np.float32)
    bind = np.zeros((CB, P, P), np.float32)
    for cc in range(P):
        gind[cc, cc // 8] = 0.125
        for cb in range(CB):
            bind[cb, cb * 32 + cc // 8, cc] = 1.0
    shared = {
        "wkqvt": np.ascontiguousarray(
            inp["lin_w"][:, :, 0].T.reshape(CB, P, 3 * C).transpose(1, 0, 2)
        ).astype(ml_dtypes.bfloat16),
        "gind": gind,
        "bind": bind,
        "ones8": np.ones((P, CB, 16), ml_dtypes.float8_e4m3),
        "onesr": np.ones((1, P), np.float32),
    }
    gnall = np.empty((P, CB, 8), np.float32)
    wsum = np.empty((P, CB, 4, C), np.float32)
    for rbi, rb in enumerate(("r1", "r2")):
        shared[f"{rb}_w1t"] = _pack_conv_w(inp[f"{rb}_c1_w"])
        shared[f"{rb}_w2t"] = _pack_conv_w(inp[f"{rb}_c2_w"])
        for ln in (1, 2):
            gnall[:, :, rbi * 4 + (ln - 1) * 2 + 0] = _pack_gn(inp[f"{rb}_gn{ln}_w"])
            gnall[:, :, rbi * 4 + (ln - 1) * 2 + 1] = _pack_gn(inp[f"{rb}_gn{ln}_b"])
            # wsum[icp, icb, j, oc] = sum_tap w[oc, ic, tap]
            w = np.asarray(inp[f"{rb}_c{ln}_w"], np.float32).sum(-1)  # (O, I)
            wsum[:, :, rbi * 2 + (ln - 1), :] = w.T.reshape(CB, P, C).transpose(1, 0, 2)
    shared["gnallt"] = gnall
    shared["wsumt"] = wsum
    if "c2b_r1" in use_bias:
        shared["r1_c2bs"] = _pack_gn(inp["r1_c2_b"])
    if "c2b_r2" in use_bias:
        shared["r2_c2bs"] = _pack_gn(inp["r2_c2_b"])
    if "linb" in use_bias:
        shared["lin_bs"] = np.ascontiguousarray(inp["lin_b"].reshape(3 * CB, P).T)

    # per-sample conv1 bias vector: t[s] + c1_b per res block -> [P, CB, 2]
    tfull = inp["t"][:, :, 0]  # (B, C)
    nb = inp["x"].shape[0]
    tv = np.empty((nb, P, CB, 2), np.float32)
    for rbi, rb in enumerate(("r1", "r2")):
        v = tfull + inp[f"{rb}_c1_b"][None, :]
        tv[:, :, :, rbi] = v.reshape(nb, CB, P).transpose(0, 2, 1)

    in_maps = []
    for c in range(NCORES):
        sl = slice(S * c, S * (c + 1))
        m = dict(shared)
        m["x"] = inp["x"][sl]
        m["tv"] = np.ascontiguousarray(tv[sl])
        in_maps.append(m)
    return in_maps


_CACHE = {}


def kernel(**inputs):
    inp = {k: np.ascontiguousarray(np.asarray(v, dtype=np.float32)) for k, v in inputs.items()}

    use_bias = []
    if np.any(inp["r1_c2_b"]):
        use_bias.append("c2b_r1")
    if np.any(inp["r2_c2_b"]):
        use_bias.append("c2b_r2")
    if np.any(inp["lin_b"]):
        use_bias.append("linb")
    use_bias = tuple(use_bias)

    if ("nc", use_bias) not in _CACHE:
        _CACHE[("nc", use_bias)] = build_program(S, use_bias)
    nc = _CACHE[("nc", use_bias)]

    in_maps = make_in_maps(inp, use_bias)
    res = _bu.run_bass_kernel_spmd(nc, in_maps, core_ids=list(range(NCORES)))
    out = np.concatenate([res.results[c]["out"] for c in range(NCORES)], axis=0)
    return out.astype(np.float32)

